# revision 1
# baseline (speedup 1.0000x reference)
"""Trainium2 Bass kernel for the dynamic-attention-block CNN (nn_DAB).

Data-parallel over batch: 8 samples -> 8 NeuronCores. Each core runs the
full per-sample network with activations resident in SBUF as padded
"frames": 128 partitions = 64 channels x 2 image halves, each half a
98x196 zero-padded row-major frame (rows -1..96 / 95..192 of the 192x192
image, cols -2..193).

Conv structure (all single 128-partition matmuls; the two image halves
ride in one instruction via block-diagonal weights):
  - 3x3 convs and dynamic depthwise convs run in fp8e4m3 DoubleRow mode:
    taps are processed in pairs (lhsT [128,2,128], rhs [128,2,N] with the
    pair dim striding between the two tap offsets), 5 passes per conv.
    Weights are pre-scaled by 16 (convs) / 64 (dw) to dodge fp8
    subnormals; the inverse scale is folded into the scalar-engine
    activation that drains PSUM.
  - 1x1 convs + channel-attention gates run in bf16: the x*att residual
    is an extra diagonal-matrix matmul accumulating into the same PSUM
    group, so no vector-engine gating pass exists at all.
  - The additive 32x32-upsampled map is folded into conv2 as one K=18
    matmul pass: 18 partitions hold the 9 tap-shifted copies of the
    upsampled map for each half, weights are the channel-summed conv2
    taps.
Activation outputs are written by the scalar engine directly in the
dtype the consumer needs (fp8 frame for the next conv, bf16 where the
gate needs precision). Residual add + output happens on DVE from f32
x+b3 staged via DMA.
"""

import sys

for _p in ("/opt/trn_rl_repo", "/root/.axon_site/_ro/pypackages"):
    if _p not in sys.path:
        sys.path.insert(0, _p)

import numpy as np
import ml_dtypes

BF16 = ml_dtypes.bfloat16
F8 = ml_dtypes.float8_e4m3

B, C, H, W = 8, 64, 192, 192
HW = H * W
FR, FC = 98, 196          # frame rows / cols per half
FF = FR * FC              # frame elems per partition
Q0 = 1 * FC + 2           # first interior frame position (row 1, col 2)
QL = 96 * FC + 194 - Q0   # sweep length covering all interior rows
TILE = 512
ALPHA = 0.1               # leaky slope
WS = 16.0                 # fp8 conv weight pre-scale
KS = 64.0                 # fp8 dw kernel pre-scale

# bias pack columns
BI_B1, BI_B2, BI_CB1, BI_CB2, BI_B3, BI_Z = range(6)

# DoubleRow tap pairing: (tap_a, tap_b) with taps t = 3*dy + dx,
# delta(t) = (dy-1)*FC + (dx-1).  5 passes cover all 9 taps; the last
# pass's second slot has zero weight (stride 0 keeps the read in-bounds).
PAIRS = [(0, 1), (3, 4), (6, 7), (2, 5), (8, None)]

_CACHE = {}


def _delta(t):
    return (t // 3 - 1) * FC + (t % 3 - 1)


def _qtiles():
    out = []
    q = Q0
    while q < Q0 + QL:
        n = min(TILE, Q0 + QL - q)
        out.append((q, n))
        q += n
    return out


def _build_nc():
    import concourse.bacc as bacc
    import concourse.mybir as mybir
    from concourse import tile

    f32 = mybir.dt.float32
    bf16 = mybir.dt.bfloat16
    f8 = mybir.dt.float8e4
    AF = mybir.ActivationFunctionType
    ALU = mybir.AluOpType
    DR = mybir.MatmulPerfMode.DoubleRow

    nc = bacc.Bacc("TRN2", target_bir_lowering=False, debug=False, num_devices=8)

    xb_d = nc.dram_tensor("xb", [128, FF], bf16, kind="ExternalInput").ap()
    x8_d = nc.dram_tensor("x8", [128, FF], f8, kind="ExternalInput").ap()
    rid_d = nc.dram_tensor("rid", [128, 128], bf16, kind="ExternalInput").ap()
    w1_d = nc.dram_tensor("w1", [128, 5, 2, 128], f8, kind="ExternalInput").ap()
    w2_d = nc.dram_tensor("w2", [128, 5, 2, 128], f8, kind="ExternalInput").ap()
    w3_d = nc.dram_tensor("w3", [128, 5, 2, 128], f8, kind="ExternalInput").ap()
    kd1_d = nc.dram_tensor("kd1", [128, 5, 2, 128], f8, kind="ExternalInput").ap()
    kd2_d = nc.dram_tensor("kd2", [128, 5, 2, 128], f8, kind="ExternalInput").ap()
    cw1_d = nc.dram_tensor("cw1", [128, 128], bf16, kind="ExternalInput").ap()
    cw2_d = nc.dram_tensor("cw2", [128, 128], bf16, kind="ExternalInput").ap()
    g1_d = nc.dram_tensor("g1", [128, 128], bf16, kind="ExternalInput").ap()
    g2_d = nc.dram_tensor("g2", [128, 128], bf16, kind="ExternalInput").ap()
    wa_d = nc.dram_tensor("wa", [18, 128], bf16, kind="ExternalInput").ap()
    af_d = nc.dram_tensor("af", [18, FF], bf16, kind="ExternalInput").ap()
    bias_d = nc.dram_tensor("bias", [128, 6], f32, kind="ExternalInput").ap()
    y_d = nc.dram_tensor("y", [C, HW], f32, kind="ExternalOutput").ap()

    # row-pair sweep tiles: 48 tiles of 2 image rows (392 frame cols)
    qt = [((2 * i + 1) * FC + 2, 2 * FC) for i in range(48)]

    from contextlib import ExitStack
    with tile.TileContext(nc) as tc, ExitStack() as ctx:
        wpool = ctx.enter_context(tc.tile_pool(name="w", bufs=1))
        fbpool = ctx.enter_context(tc.tile_pool(name="fb", bufs=2))
        f8pool = ctx.enter_context(tc.tile_pool(name="f8", bufs=3))
        t1p = ctx.enter_context(tc.tile_pool(name="t1", bufs=4))
        outp = ctx.enter_context(tc.tile_pool(name="outp", bufs=4))
        psA = ctx.enter_context(tc.tile_pool(name="psA", bufs=4, space="PSUM"))
        psB = ctx.enter_context(tc.tile_pool(name="psB", bufs=4, space="PSUM"))

        # ---- weights / constants to SBUF ----
        w1 = wpool.tile([128, 5, 2, 128], f8, tag="w1")
        w2 = wpool.tile([128, 5, 2, 128], f8, tag="w2")
        w3 = wpool.tile([128, 5, 2, 128], f8, tag="w3")
        kd1 = wpool.tile([128, 5, 2, 128], f8, tag="kd1")
        kd2 = wpool.tile([128, 5, 2, 128], f8, tag="kd2")
        cw1 = wpool.tile([128, 128], bf16, tag="cw1")
        cw2 = wpool.tile([128, 128], bf16, tag="cw2")
        g1 = wpool.tile([128, 128], bf16, tag="g1")
        g2 = wpool.tile([128, 128], bf16, tag="g2")
        rid = wpool.tile([128, 128], bf16, tag="rid")     # diag(WS) residual
        wa = wpool.tile([18, 128], bf16, tag="wa")
        af = wpool.tile([18, FF], bf16, tag="af")
        bias = wpool.tile([128, 6], f32, tag="bias")
        for t, d in ((w1, w1_d), (w2, w2_d), (w3, w3_d), (kd1, kd1_d),
                     (kd2, kd2_d), (cw1, cw1_d), (cw2, cw2_d), (g1, g1_d),
                     (g2, g2_d), (rid, rid_d), (wa, wa_d),
                     (bias, bias_d)):
            nc.gpsimd.dma_start(out=t[...], in_=d)
        nc.scalar.dma_start(out=af[:, :], in_=af_d)

        def cold(col):
            return bias[:, col:col + 1]

        # ---- input frames (host pre-padded); fp8 frame first ----
        Xb = fbpool.tile([128, FF], bf16, tag="fb")
        X8 = f8pool.tile([128, FF], f8, tag="f8")
        O1 = f8pool.tile([128, FF], f8, tag="f8")
        O2 = f8pool.tile([128, FF], f8, tag="f8")
        O3b = fbpool.tile([128, FF], bf16, tag="fb")
        qs = (nc.sync, nc.scalar, nc.gpsimd)
        step8 = (FF + 2) // 3
        for k in range(3):
            c0, c1 = k * step8, min((k + 1) * step8, FF)
            qs[k].dma_start(out=X8[:, c0:c1], in_=x8_d[:, c0:c1])
        nchunk = 6
        step = (FF + nchunk - 1) // nchunk
        for k in range(nchunk):
            c0, c1 = k * step, min((k + 1) * step, FF)
            qs[k % 3].dma_start(out=Xb[:, c0:c1], in_=xb_d[:, c0:c1])

        # ---- PE warmup: throwaway matmuls while the input DMAs stream
        # in; keeps the p-state ramp finished before real work ----
        wrm = wpool.tile([128, TILE], bf16, tag="wrm")
        nc.vector.memset(wrm[:, :], 0.0)
        pw = psA.tile([128, TILE], f32, tag="psA")
        for _ in range(14):
            nc.tensor.matmul(pw[:, :], wrm[:, 0:128], wrm[:, :],
                             start=True, stop=True, skip_group_check=True)

        def v3(m):
            return m[:, :].rearrange("p (a b) -> p a b", b=FC)

        # one-time pad zeroing for frame buffers not filled by host DMA.
        # Interior writes never touch pads again, so pads stay zero across
        # all later reuses of these pool buffers.
        for m in (O1, O2, O3b):
            mv = v3(m)
            nc.gpsimd.memset(mv[0:64, 0, :], 0.0)
            nc.gpsimd.memset(mv[64:128, FR - 1, :], 0.0)
            nc.gpsimd.memset(mv[:, :, 0:2], 0.0)
            nc.gpsimd.memset(mv[:, :, FC - 2:FC], 0.0)

        def halo(m):
            mv = v3(m)
            nc.gpsimd.dma_start(out=mv[0:64, FR - 1, :], in_=mv[64:128, 1, :])
            nc.gpsimd.dma_start(out=mv[64:128, 0, :], in_=mv[0:64, 96, :])

        def dr_rhs(m8, q, n, pair):
            ta, tb = pair
            base = q + _delta(ta)
            stride = 0 if tb is None else _delta(tb) - _delta(ta)
            n = min(n, FF - base - max(stride, 0))
            r = m8[:, base:base + 1].copy()
            r.ap[1] = [stride, 2]
            r.ap.append([1, n])
            return r, n

        def conv_dr(ps, wsb, m8, q, n):
            # P0 (top-left taps) never clamps, so it is the start pass and
            # always covers the full tile; clamped later passes only lose
            # tail columns that are pad positions, never emitted.
            for p in range(5):
                rhs, np_ = dr_rhs(m8, q, n, PAIRS[p])
                nc.tensor.matmul(ps[:, :np_], wsb[:, p, :, :], rhs,
                                 start=(p == 0), stop=(p == 4), perf_mode=DR,
                                 skip_group_check=True)

        def act_out(dst, src_ps, q, n, func, **kw):
            # interior-only write: rows of the pair, cols 2:194
            r = q // FC
            dv = v3(dst)[:, r:r + 2, 2:194]
            sv = src_ps[:, :n].rearrange("p (a b) -> p a b", b=FC)[:, :, 0:192]
            nc.scalar.activation(dv, sv, func, **kw)

        def da_stage(inb, in8, kdsb, cwsb, gsb, cb_col, out8, dve_tail=False):
            # software-pipelined by one tile: PE issues dw(j) before the
            # 1x1+gate of tile j-1 so the in-order PE queue never stalls
            # behind the scalar engine's t1 prelu.
            def tail(prev):
                t1, q, n = prev
                pb = psB.tile([128, 2 * FC], f32, tag="psB")
                nc.tensor.matmul(pb[:, :n], cwsb[:, :], t1[:, :n],
                                 start=True, stop=False, skip_group_check=True)
                nc.tensor.matmul(pb[:, :n], gsb[:, :], inb[:, q:q + n],
                                 start=False, stop=True, skip_group_check=True)
                if dve_tail:
                    r = q // FC
                    dv = v3(out8)[:, r:r + 2, 2:194]
                    sv = pb[:, :n].rearrange("p (a b) -> p a b", b=FC)[:, :, 0:192]
                    t4 = t1p.tile([128, 2 * FC], bf16, tag="t4")
                    t4v = t4[:, :n].rearrange("p (a b) -> p a b", b=FC)[:, :, 0:192]
                    nc.vector.tensor_scalar(t4v, sv, cold(cb_col), None,
                                            op0=ALU.add)
                    nc.vector.scalar_tensor_tensor(dv, t4v, ALPHA, t4v,
                                                   op0=ALU.mult, op1=ALU.max)
                else:
                    act_out(out8, pb, q, n, AF.Prelu, bias=cold(cb_col),
                            alpha=ALPHA)
            prev = None
            for (q, n) in qt:
                pa = psA.tile([128, 2 * FC], f32, tag="psA")
                conv_dr(pa, kdsb, in8, q, n)
                if prev is not None:
                    tail(prev)
                t1 = t1p.tile([128, 2 * FC], bf16, tag="t1")
                nc.scalar.activation(t1[:, :n], pa[:, :n], AF.Prelu,
                                     scale=1.0 / KS, bias=cold(BI_Z), alpha=ALPHA)
                prev = (t1, q, n)
            tail(prev)
            halo(out8)

        # ---- network ----
        da_stage(Xb, X8, kd1, cw1, g1, BI_CB1, O1)

        # conv1 -> prelu -> fp8 frame
        for (q, n) in qt:
            pa = psA.tile([128, 2 * FC], f32, tag="psA")
            conv_dr(pa, w1, O1, q, n)
            act_out(O2, pa, q, n, AF.Prelu, scale=1.0 / WS, bias=cold(BI_B1),
                    alpha=ALPHA)
        halo(O2)

        # conv2 (+ additive map as a K=18 pass) -> identity+bias -> bf16+fp8
        O38 = f8pool.tile([128, FF], f8, tag="f8")
        for (q, n) in qt:
            pa = psA.tile([128, 2 * FC], f32, tag="psA")
            for p in range(5):
                rhs, np_ = dr_rhs(O2, q, n, PAIRS[p])
                nc.tensor.matmul(pa[:, :np_], w2[:, p, :, :], rhs,
                                 start=(p == 0), stop=False, perf_mode=DR,
                                 skip_group_check=True)
            nc.tensor.matmul(pa[:, :n], wa[:, :], af[:, q:q + n],
                             start=False, stop=True, skip_group_check=True)
            act_out(O3b, pa, q, n, AF.Identity, scale=1.0 / WS, bias=cold(BI_B2))
            nc.vector.tensor_copy(O38[:, q:q + n], O3b[:, q:q + n])
        halo(O3b)
        halo(O38)

        O4 = f8pool.tile([128, FF], f8, tag="f8")
        da_stage(O3b, O38, kd2, cw2, g2, BI_CB2, O4, dve_tail=True)

        # ---- conv3 + residual: x (bf16, scaled by WS via diag weights)
        # and WS*b3 accumulate straight into PSUM; Act drains with 1/WS ----
        for j, (q, n) in enumerate(qt):
            pa = psA.tile([128, 2 * FC], f32, tag="psA")
            conv_dr(pa, w3, O4, q, n)
            nc.tensor.matmul(pa[:, :n], rid[:, :], Xb[:, q:q + n],
                             start=False, stop=True, skip_group_check=True)
            ot = outp.tile([128, 2, 192], f32, tag="ot")
            nc.scalar.activation(
                ot[:, :, :],
                pa[:, :n].rearrange("p (a b) -> p a b", b=FC)[:, :, 0:192],
                AF.Identity, scale=1.0 / WS, bias=cold(BI_B3))
            r0 = q // FC - 1  # image row of the pair
            qs[j % 3].dma_start(
                out=y_d[:, r0 * 192:(r0 + 2) * 192]
                .rearrange("p (r c) -> p r c", c=192),
                in_=ot[0:64, :, :])
            qs[(j + 1) % 3].dma_start(
                out=y_d[:, (96 + r0) * 192:(96 + r0 + 2) * 192]
                .rearrange("p (r c) -> p r c", c=192),
                in_=ot[64:128, :, :])

    nc.compile()
    return nc


def _pad_frame(xb, dtype):
    """(64,192,192) fp32 -> (128, FR*FC) dual-half padded frame."""
    fr = np.zeros((128, FR, FC), np.float32)
    fr[0:64, 1:97, 2:194] = xb[:, 0:96, :]
    fr[0:64, 97, 2:194] = xb[:, 96, :]
    fr[64:128, 1:97, 2:194] = xb[:, 96:192, :]
    fr[64:128, 0, 2:194] = xb[:, 95, :]
    return np.ascontiguousarray(fr.reshape(128, FF)).astype(dtype)


def _leaky_np(v):
    return np.where(v >= 0, v, ALPHA * v)


def _host_precompute(x, d, p):
    """Build per-core input maps. p: dict of raw weight arrays."""
    d = d.astype(np.float64)
    kern = {}
    att = {}
    for i in (1, 2):
        kw1, kw2 = p[f'da{i}_kw1'].astype(np.float64), p[f'da{i}_kw2'].astype(np.float64)
        ca1, ca2 = p[f'da{i}_ca1'].astype(np.float64), p[f'da{i}_ca2'].astype(np.float64)
        kern[i] = _leaky_np(d @ kw1.T) @ kw2.T          # (B, 576) [c*9+t]
        z = _leaky_np(d @ ca1.T) @ ca2.T
        att[i] = 1.0 / (1.0 + np.exp(-z))               # (B, 64)
    a32 = _leaky_np(d @ p['add_w1'].astype(np.float64).T) @ \
        p['add_w2'].astype(np.float64).T                # (B, 1024)

    cidx = np.arange(128) % 64
    hidx = np.arange(128) // 64

    def convw_dr(w):
        # (O, C, 3, 3) fp32 -> [128, 5, 2, 128] f8 block-diag DoubleRow taps
        wq = (w.astype(np.float32) * WS).astype(F8).astype(np.float32)
        wt = wq.transpose(1, 2, 3, 0).reshape(64, 9, 64)  # [c, t, o]
        out = np.zeros((128, 5, 2, 128), np.float32)
        for pi, (ta, tb) in enumerate(PAIRS):
            blk = np.zeros((64, 2, 64), np.float32)
            blk[:, 0, :] = wt[:, ta, :]
            if tb is not None:
                blk[:, 1, :] = wt[:, tb, :]
            out[0:64, pi, :, 0:64] = blk
            out[64:128, pi, :, 64:128] = blk
        return np.ascontiguousarray(out).astype(F8)

    def cw_bd(w):
        # (O, C) -> [128, 128] bf16 block-diag: [p, o]
        out = np.zeros((128, 128), np.float32)
        out[0:64, 0:64] = w.T
        out[64:128, 64:128] = w.T
        return np.ascontiguousarray(out).astype(BF16)

    w1 = convw_dr(p['conv1_w'])
    w2 = convw_dr(p['conv2_w'])
    w3 = convw_dr(p['conv3_w'])
    cw1 = cw_bd(p['da1_cw'])
    cw2 = cw_bd(p['da2_cw'])

    # additive-map conv weights: wa[(h,t), o_col] = WS * sum_c conv2_w[o,c,t]
    w2sum = p['conv2_w'].astype(np.float64).sum(axis=1).reshape(64, 9)  # [o, t]
    wa = np.zeros((18, 128), np.float32)
    for h in range(2):
        for t in range(9):
            wa[h * 9 + t, h * 64:(h + 1) * 64] = WS * w2sum[:, t]
    wa = np.ascontiguousarray(wa).astype(BF16)

    rid = np.ascontiguousarray(_diag128(np.full(128, WS, np.float32))).astype(BF16)

    maps = []
    for b in range(B):
        kd = {}
        for i in (1, 2):
            kc = (kern[i][b].reshape(64, 9).astype(np.float32) * KS) \
                .astype(F8).astype(np.float32)           # [c, t]
            kdl = np.zeros((128, 5, 2, 128), np.float32)
            for pi, (ta, tb) in enumerate(PAIRS):
                kdl[np.arange(128), pi, 0, np.arange(128)] = kc[cidx, ta]
                if tb is not None:
                    kdl[np.arange(128), pi, 1, np.arange(128)] = kc[cidx, tb]
            kd[i] = np.ascontiguousarray(kdl).astype(F8)
        g = {i: np.ascontiguousarray(_diag128(att[i][b][cidx])).astype(BF16)
             for i in (1, 2)}
        bias = np.zeros((128, 6), np.float32)
        bias[:, BI_B1] = p['conv1_b'][cidx]
        bias[:, BI_B2] = p['conv2_b'][cidx]
        bias[:, BI_CB1] = p['da1_cb'][cidx]
        bias[:, BI_CB2] = p['da2_cb'][cidx]
        bias[:, BI_B3] = p['conv3_b'][cidx]

        # additive map frames: 18 partitions = 2 halves x 9 tap shifts
        a = a32[b].astype(np.float32).reshape(32, 32)
        aup = a[np.arange(192) // 6][:, np.arange(192) // 6]  # (192,192)
        afr = np.zeros((2, FF), np.float32)
        fr0 = np.zeros((FR, FC), np.float32)
        fr0[1:97, 2:194] = aup[0:96]
        fr0[97, 2:194] = aup[96]
        afr[0] = fr0.reshape(FF)
        fr1 = np.zeros((FR, FC), np.float32)
        fr1[1:97, 2:194] = aup[96:192]
        fr1[0, 2:194] = aup[95]
        afr[1] = fr1.reshape(FF)
        af = np.zeros((18, FF), np.float32)
        for h in range(2):
            for t in range(9):
                dlt = _delta(t)
                src = afr[h]
                dst = np.zeros(FF, np.float32)
                if dlt >= 0:
                    dst[:FF - dlt] = src[dlt:]
                else:
                    dst[-dlt:] = src[:FF + dlt]
                af[h * 9 + t] = dst
        maps.append(dict(
            xb=_pad_frame(x[b], BF16),
            x8=_pad_frame(x[b], F8),
            rid=rid,
            w1=w1, w2=w2, w3=w3, kd1=kd[1], kd2=kd[2], cw1=cw1, cw2=cw2,
            g1=g[1], g2=g[2], wa=wa,
            af=np.ascontiguousarray(af).astype(BF16),
            bias=bias))
    return maps


def _diag128(v):
    out = np.zeros((128, 128), np.float32)
    out[np.arange(128), np.arange(128)] = v
    return out


def kernel(**inputs):
    from concourse.bass_utils import run_bass_kernel_spmd

    x = np.asarray(inputs['x'], np.float32)
    d = np.asarray(inputs['d'], np.float32)
    in_maps = _host_precompute(x, d, inputs)

    if 'nc' not in _CACHE:
        _CACHE['nc'] = _build_nc()
    nc = _CACHE['nc']

    try:
        res = run_bass_kernel_spmd(nc, in_maps, list(range(B)))
    except Exception:
        # transient NRT_EXEC_UNIT_UNRECOVERABLE observed on back-to-back
        # runs; a single retry is free and often clears it
        res = run_bass_kernel_spmd(nc, in_maps, list(range(B)))
    out = np.stack([np.asarray(res.results[i]['y'], np.float32).reshape(C, H, W)
                    for i in range(B)])
    return out



# revision 6
# speedup vs baseline: 1.1449x; 1.1449x over previous
"""Trainium2 Bass kernel for the dynamic-attention-block CNN (nn_DAB).

Data-parallel over batch: 8 samples -> 8 NeuronCores. Each core runs the
full per-sample network with activations resident in SBUF as padded
"frames": 128 partitions = 64 channels x 2 image halves, each half a
98x196 zero-padded row-major frame (rows -1..96 / 95..192 of the 192x192
image, cols -2..193).

Conv structure (all single 128-partition matmuls; the two image halves
ride in one instruction via block-diagonal weights):
  - 3x3 convs and dynamic depthwise convs run in fp8e4m3 DoubleRow mode:
    taps are processed in pairs (lhsT [128,2,128], rhs [128,2,N] with the
    pair dim striding between the two tap offsets), 5 passes per conv.
    Weights are pre-scaled by 16 (convs) / 64 (dw) to dodge fp8
    subnormals; the inverse scale is folded into the engine op that
    drains PSUM.
  - 1x1 convs + channel-attention gates run in bf16: the x*att residual
    is an extra diagonal-matrix matmul accumulating into the same PSUM
    group, so no vector-engine gating pass exists at all.
  - The additive 32x32-upsampled map is folded into conv2 as one fp8
    DoubleRow matmul pass (stride-0 pair, second slot zero): 18
    partitions hold the 9 tap-shifted copies of the upsampled map for
    each half, weights are the channel-summed conv2 taps.

The five stages are software-pipelined ACROSS stage boundaries: stage k
visits tiles in an order rotated by 2(k-1), and the halo rows (the only
cross-half data) are DMA'd as soon as their source tiles (47 and 0)
drain, so the next stage's convs never wait on the previous stage's
tail. Engine balance per tile: PE does all matmuls; DVE drains the two
dw-conv PSUMs (prelu, scale folded into the following 1x1 weights) and
conv2's PSUM; Act drains the two da outputs, conv1 and conv3; GPSIMD
makes conv2's fp8 frame copy.
"""

import sys

for _p in ("/opt/trn_rl_repo", "/root/.axon_site/_ro/pypackages"):
    if _p not in sys.path:
        sys.path.insert(0, _p)

import numpy as np
import ml_dtypes

BF16 = ml_dtypes.bfloat16
F8 = ml_dtypes.float8_e4m3

B, C, H, W = 8, 64, 192, 192
HW = H * W
FR, FC = 98, 196          # frame rows / cols per half
FF = FR * FC              # frame elems per partition
ALPHA = 0.1               # leaky slope
WS = 16.0                 # fp8 conv weight pre-scale
KS = 64.0                 # fp8 dw kernel pre-scale

# bias pack columns (B1/B2/B3 pre-scaled by WS for the DVE drains)
BI_B1W, BI_B2W, BI_CB1, BI_CB2, BI_B3W = range(5)

# DoubleRow tap pairing: (tap_a, tap_b) with taps t = 3*dy + dx,
# delta(t) = (dy-1)*FC + (dx-1).  5 passes cover all 9 taps; the last
# pass's second slot has zero weight (stride 0 keeps the read in-bounds).
PAIRS = [(0, 1), (3, 4), (6, 7), (2, 5), (8, None)]

# packed fp8 weight slots in wpack8
WP_W1, WP_W2, WP_W3, WP_KD1, WP_KD2 = range(5)
# packed bf16 weight slots in wpackb
WB_CW1, WB_CW2, WB_G1, WB_G2, WB_RID = range(5)

NTILE = 48
ROT = 2                   # per-stage tile-order rotation

_CACHE = {}


def _delta(t):
    return (t // 3 - 1) * FC + (t % 3 - 1)


def _build_nc():
    import concourse.bacc as bacc
    import concourse.mybir as mybir
    from concourse import tile

    f32 = mybir.dt.float32
    bf16 = mybir.dt.bfloat16
    f8 = mybir.dt.float8e4
    AF = mybir.ActivationFunctionType
    ALU = mybir.AluOpType
    DR = mybir.MatmulPerfMode.DoubleRow

    nc = bacc.Bacc("TRN2", target_bir_lowering=False, debug=False, num_devices=8)

    xb_d = nc.dram_tensor("xb", [128, FF], bf16, kind="ExternalInput").ap()
    x8_d = nc.dram_tensor("x8", [128, FF], f8, kind="ExternalInput").ap()
    wp8_d = nc.dram_tensor("wp8", [128, 25, 2, 128], f8, kind="ExternalInput").ap()
    wpb_d = nc.dram_tensor("wpb", [128, 5, 128], bf16, kind="ExternalInput").ap()
    wa_d = nc.dram_tensor("wa", [18, 2, 128], f8, kind="ExternalInput").ap()
    af_d = nc.dram_tensor("af", [18, FF], f8, kind="ExternalInput").ap()
    bias_d = nc.dram_tensor("bias", [128, 5], f32, kind="ExternalInput").ap()
    y_d = nc.dram_tensor("y", [C, HW], f32, kind="ExternalOutput").ap()

    # row-pair sweep tiles: 48 tiles of 2 image rows (392 frame cols)
    qt = [((2 * i + 1) * FC + 2, 2 * FC) for i in range(NTILE)]

    from contextlib import ExitStack
    with tile.TileContext(nc) as tc, ExitStack() as ctx:
        wpool = ctx.enter_context(tc.tile_pool(name="w", bufs=1))
        fbpool = ctx.enter_context(tc.tile_pool(name="fb", bufs=2))
        f8pool = ctx.enter_context(tc.tile_pool(name="f8", bufs=3))
        t1p = ctx.enter_context(tc.tile_pool(name="t1", bufs=4))
        outp = ctx.enter_context(tc.tile_pool(name="outp", bufs=4))
        psA = ctx.enter_context(tc.tile_pool(name="psA", bufs=4, space="PSUM"))
        psB = ctx.enter_context(tc.tile_pool(name="psB", bufs=4, space="PSUM"))

        # ---- weights / constants to SBUF (batched DMAs) ----
        wp8 = wpool.tile([128, 25, 2, 128], f8, tag="wp8")
        wpb = wpool.tile([128, 5, 128], bf16, tag="wpb")
        wa = wpool.tile([18, 2, 128], f8, tag="wa")
        af = wpool.tile([18, FF], f8, tag="af")
        bias = wpool.tile([128, 5], f32, tag="bias")
        nc.sync.dma_start(out=wp8[...], in_=wp8_d)
        nc.gpsimd.dma_start(out=wpb[...], in_=wpb_d)
        nc.gpsimd.dma_start(out=wa[...], in_=wa_d)
        nc.gpsimd.dma_start(out=bias[...], in_=bias_d)
        nc.scalar.dma_start(out=af[...], in_=af_d)

        def w8(slot):
            return wp8[:, 5 * slot:5 * slot + 5, :, :]

        def wb(slot):
            return wpb[:, slot, :]

        def cold(col):
            return bias[:, col:col + 1]

        # ---- input frames (host pre-padded); fp8 frame first ----
        Xb = fbpool.tile([128, FF], bf16, tag="fb")
        X8 = f8pool.tile([128, FF], f8, tag="f8")
        O1 = f8pool.tile([128, FF], f8, tag="f8")
        O2 = f8pool.tile([128, FF], f8, tag="f8")
        O3b = fbpool.tile([128, FF], bf16, tag="fb")
        qs = (nc.sync, nc.scalar, nc.gpsimd)
        step8 = (FF + 2) // 3
        for k in range(3):
            c0, c1 = k * step8, min((k + 1) * step8, FF)
            qs[k].dma_start(out=X8[:, c0:c1], in_=x8_d[:, c0:c1])
        nchunk = 6
        step = (FF + nchunk - 1) // nchunk
        for k in range(nchunk):
            c0, c1 = k * step, min((k + 1) * step, FF)
            qs[k % 3].dma_start(out=Xb[:, c0:c1], in_=xb_d[:, c0:c1])

        # ---- PE warmup: throwaway matmuls while the input DMAs stream
        # in; keeps the p-state ramp finished before real work ----
        wrm = wpool.tile([128, 512], bf16, tag="wrm")
        nc.vector.memset(wrm[:, :], 0.0)
        pw = psA.tile([128, 512], f32, tag="psA")
        for _ in range(14):
            nc.tensor.matmul(pw[:, :], wrm[:, 0:128], wrm[:, :],
                             start=True, stop=True, skip_group_check=True)

        def v3(m):
            return m[:, :].rearrange("p (a b) -> p a b", b=FC)

        # one-time pad zeroing for frame buffers not filled by host DMA.
        # Interior writes never touch pads again, so pads stay zero across
        # all later reuses of these pool buffers.
        for m in (O1, O2, O3b):
            mv = v3(m)
            nc.gpsimd.memset(mv[0:64, 0, :], 0.0)
            nc.gpsimd.memset(mv[64:128, FR - 1, :], 0.0)
            nc.gpsimd.memset(mv[:, :, 0:2], 0.0)
            nc.gpsimd.memset(mv[:, :, FC - 2:FC], 0.0)

        def halo_a(m):
            # half1 top halo row (img 95) <- half0 frame row 96, src tile 47
            mv = v3(m)
            nc.gpsimd.dma_start(out=mv[64:128, 0, :], in_=mv[0:64, 96, :])

        def halo_b(m):
            # half0 bottom halo row (img 96) <- half1 frame row 1, src tile 0
            mv = v3(m)
            nc.gpsimd.dma_start(out=mv[0:64, FR - 1, :], in_=mv[64:128, 1, :])

        def maybe_halo(t, frames):
            if t == 47:
                for m in frames:
                    halo_a(m)
            elif t == 0:
                for m in frames:
                    halo_b(m)

        def order(stage):
            s = (ROT * stage) % NTILE
            return [(s + i) % NTILE for i in range(NTILE)]

        def dr_rhs(m8, q, n, pair):
            ta, tb = pair
            base = q + _delta(ta)
            stride = 0 if tb is None else _delta(tb) - _delta(ta)
            n = min(n, FF - base - max(stride, 0))
            r = m8[:, base:base + 1].copy()
            r.ap[1] = [stride, 2]
            r.ap.append([1, n])
            return r, n

        def conv_dr(ps, wsb, m8, q, n, stop=True):
            # P0 (top-left taps) never clamps, so it is the start pass and
            # always covers the full tile; clamped later passes only lose
            # tail columns that are pad positions, never emitted.
            for p in range(5):
                rhs, np_ = dr_rhs(m8, q, n, PAIRS[p])
                nc.tensor.matmul(ps[:, :np_], wsb[:, p, :, :], rhs,
                                 start=(p == 0), stop=(stop and p == 4),
                                 perf_mode=DR, skip_group_check=True)

        def iview(dst, q):
            # interior-only view: rows of the pair, cols 2:194
            r = q // FC
            return v3(dst)[:, r:r + 2, 2:194]

        def pview(src_ps, n):
            return src_ps[:, :n].rearrange("p (a b) -> p a b", b=FC)[:, :, 0:192]

        def da_stage(stage, inb, in8, kd_slot, cw_slot, g_slot, cb_col, out8):
            # software-pipelined by one tile: PE issues dw(j) before the
            # 1x1+gate of tile j-1 so the in-order PE queue never stalls
            # behind the Act t1 prelu.
            kd, cw, g = w8(kd_slot), wb(cw_slot), wb(g_slot)

            def tail(prev):
                t1, q, n, t = prev
                pb = psB.tile([128, 2 * FC], f32, tag="psB")
                nc.tensor.matmul(pb[:, :n], cw, t1[:, :n],
                                 start=True, stop=False, skip_group_check=True)
                nc.tensor.matmul(pb[:, :n], g, inb[:, q:q + n],
                                 start=False, stop=True, skip_group_check=True)
                nc.scalar.activation(iview(out8, q), pview(pb, n), AF.Prelu,
                                     bias=cold(cb_col), alpha=ALPHA)
                maybe_halo(t, (out8,))

            prev = None
            for t in order(stage):
                q, n = qt[t]
                pa = psA.tile([128, 2 * FC], f32, tag="psA")
                conv_dr(pa, kd, in8, q, n)
                if prev is not None:
                    tail(prev)
                # t1 = prelu(psA); the KS dw-weight scale rides along
                # (prelu is positively homogeneous) and is divided out of
                # the 1x1 weights on the host.
                t1 = t1p.tile([128, 2 * FC], bf16, tag="t1")
                nc.scalar.activation(t1[:, :n], pa[:, :n], AF.Prelu,
                                     alpha=ALPHA)
                prev = (t1, q, n, t)
            tail(prev)

        # ---- network ----
        da_stage(0, Xb, X8, WP_KD1, WB_CW1, WB_G1, BI_CB1, O1)

        # conv1 -> prelu -> fp8 frame (DVE drain: two-scalar tensor_scalar
        # to a bf16 staging tile, then an SBUF-only prelu into the frame;
        # scalar_tensor_tensor cannot read PSUM)
        for t in order(1):
            q, n = qt[t]
            pa = psA.tile([128, 2 * FC], f32, tag="psA")
            conv_dr(pa, w8(WP_W1), O1, q, n)
            tm = t1p.tile([128, 2 * FC], bf16, tag="t1")
            tv = tm[:, :n].rearrange("p (a b) -> p a b", b=FC)[:, :, 0:192]
            nc.vector.tensor_scalar(tv, pview(pa, n), cold(BI_B1W), 1.0 / WS,
                                    op0=ALU.add, op1=ALU.mult)
            nc.vector.scalar_tensor_tensor(iview(O2, q), tv, ALPHA, tv,
                                           op0=ALU.mult, op1=ALU.max)
            maybe_halo(t, (O2,))

        # conv2 (+ additive map as a stride-0 fp8 DR pass) -> identity+bias
        # -> bf16 (DVE drain) + fp8 copy (GPSIMD)
        O38 = f8pool.tile([128, FF], f8, tag="f8")
        for t in order(2):
            q, n = qt[t]
            pa = psA.tile([128, 2 * FC], f32, tag="psA")
            conv_dr(pa, w8(WP_W2), O2, q, n, stop=False)
            r = af[:, q:q + 1].copy()
            r.ap[1] = [0, 2]
            r.ap.append([1, n])
            nc.tensor.matmul(pa[:, :n], wa[:, :, :], r,
                             start=False, stop=True, perf_mode=DR,
                             skip_group_check=True)
            nc.vector.tensor_scalar(iview(O3b, q), pview(pa, n),
                                    cold(BI_B2W), 1.0 / WS,
                                    op0=ALU.add, op1=ALU.mult)
            nc.gpsimd.tensor_copy(O38[:, q:q + n], O3b[:, q:q + n])
            maybe_halo(t, (O38,))

        O4 = f8pool.tile([128, FF], f8, tag="f8")
        da_stage(3, O3b, O38, WP_KD2, WB_CW2, WB_G2, BI_CB2, O4)

        # ---- conv3 + residual: x (bf16, scaled by WS via diag weights)
        # and WS*b3 accumulate straight into PSUM; DVE drains with 1/WS ----
        for j, t in enumerate(order(4)):
            q, n = qt[t]
            pa = psA.tile([128, 2 * FC], f32, tag="psA")
            conv_dr(pa, w8(WP_W3), O4, q, n, stop=False)
            nc.tensor.matmul(pa[:, :n], wb(WB_RID), Xb[:, q:q + n],
                             start=False, stop=True, skip_group_check=True)
            ot = outp.tile([128, 2, 192], f32, tag="ot")
            nc.vector.tensor_scalar(ot[:, :, :], pview(pa, n), cold(BI_B3W),
                                    1.0 / WS, op0=ALU.add, op1=ALU.mult)
            r0 = q // FC - 1  # image row of the pair
            qs[j % 3].dma_start(
                out=y_d[:, r0 * 192:(r0 + 2) * 192]
                .rearrange("p (r c) -> p r c", c=192),
                in_=ot[0:64, :, :])
            qs[(j + 1) % 3].dma_start(
                out=y_d[:, (96 + r0) * 192:(96 + r0 + 2) * 192]
                .rearrange("p (r c) -> p r c", c=192),
                in_=ot[64:128, :, :])

    nc.compile()
    return nc


def _pad_frame(xb, dtype):
    """(64,192,192) fp32 -> (128, FR*FC) dual-half padded frame."""
    fr = np.zeros((128, FR, FC), np.float32)
    fr[0:64, 1:97, 2:194] = xb[:, 0:96, :]
    fr[0:64, 97, 2:194] = xb[:, 96, :]
    fr[64:128, 1:97, 2:194] = xb[:, 96:192, :]
    fr[64:128, 0, 2:194] = xb[:, 95, :]
    return np.ascontiguousarray(fr.reshape(128, FF)).astype(dtype)


def _leaky_np(v):
    return np.where(v >= 0, v, ALPHA * v)


def _host_precompute(x, d, p):
    """Build per-core input maps. p: dict of raw weight arrays."""
    d = d.astype(np.float64)
    kern = {}
    att = {}
    for i in (1, 2):
        kw1, kw2 = p[f'da{i}_kw1'].astype(np.float64), p[f'da{i}_kw2'].astype(np.float64)
        ca1, ca2 = p[f'da{i}_ca1'].astype(np.float64), p[f'da{i}_ca2'].astype(np.float64)
        kern[i] = _leaky_np(d @ kw1.T) @ kw2.T          # (B, 576) [c*9+t]
        z = _leaky_np(d @ ca1.T) @ ca2.T
        att[i] = 1.0 / (1.0 + np.exp(-z))               # (B, 64)
    a32 = _leaky_np(d @ p['add_w1'].astype(np.float64).T) @ \
        p['add_w2'].astype(np.float64).T                # (B, 1024)

    cidx = np.arange(128) % 64

    def convw_dr(w):
        # (O, C, 3, 3) fp32 -> [128, 5, 2, 128] f8 block-diag DoubleRow taps
        wq = (w.astype(np.float32) * WS).astype(F8).astype(np.float32)
        wt = wq.transpose(1, 2, 3, 0).reshape(64, 9, 64)  # [c, t, o]
        out = np.zeros((128, 5, 2, 128), np.float32)
        for pi, (ta, tb) in enumerate(PAIRS):
            blk = np.zeros((64, 2, 64), np.float32)
            blk[:, 0, :] = wt[:, ta, :]
            if tb is not None:
                blk[:, 1, :] = wt[:, tb, :]
            out[0:64, pi, :, 0:64] = blk
            out[64:128, pi, :, 64:128] = blk
        return out.astype(F8)

    def cw_bd(w, scale=1.0):
        # (O, C) -> [128, 128] block-diag: [p, o]
        out = np.zeros((128, 128), np.float32)
        out[0:64, 0:64] = w.T * scale
        out[64:128, 64:128] = w.T * scale
        return out

    # fp8 packed conv/dw weights (per-sample kd slots filled below)
    w1 = convw_dr(p['conv1_w'])
    w2 = convw_dr(p['conv2_w'])
    w3 = convw_dr(p['conv3_w'])
    # 1/KS folds the dw pre-scale out of the un-scaled DVE t1 prelu
    cw1 = cw_bd(p['da1_cw'], 1.0 / KS)
    cw2 = cw_bd(p['da2_cw'], 1.0 / KS)

    # additive-map conv weights: wa[(h,t), 0, o_col] = WS * sum_c w2[o,c,t]
    w2sum = p['conv2_w'].astype(np.float64).sum(axis=1).reshape(64, 9)  # [o, t]
    wa = np.zeros((18, 2, 128), np.float32)
    for h in range(2):
        for t in range(9):
            wa[h * 9 + t, 0, h * 64:(h + 1) * 64] = WS * w2sum[:, t]
    wa = wa.astype(F8)

    rid = _diag128(np.full(128, WS, np.float32))

    maps = []
    for b in range(B):
        kd = {}
        for i in (1, 2):
            kc = (kern[i][b].reshape(64, 9).astype(np.float32) * KS) \
                .astype(F8).astype(np.float32)           # [c, t]
            kdl = np.zeros((128, 5, 2, 128), np.float32)
            for pi, (ta, tb) in enumerate(PAIRS):
                kdl[np.arange(128), pi, 0, np.arange(128)] = kc[cidx, ta]
                if tb is not None:
                    kdl[np.arange(128), pi, 1, np.arange(128)] = kc[cidx, tb]
            kd[i] = kdl.astype(F8)
        g = {i: _diag128(att[i][b][cidx]) for i in (1, 2)}
        wp8 = np.concatenate(
            [w1, w2, w3, kd[1], kd[2]], axis=1).reshape(128, 25, 2, 128)
        wpb = np.stack(
            [cw1, cw2, g[1], g[2], rid], axis=1).astype(BF16)
        bias = np.zeros((128, 5), np.float32)
        bias[:, BI_B1W] = WS * p['conv1_b'][cidx]
        bias[:, BI_B2W] = WS * p['conv2_b'][cidx]
        bias[:, BI_CB1] = p['da1_cb'][cidx]
        bias[:, BI_CB2] = p['da2_cb'][cidx]
        bias[:, BI_B3W] = WS * p['conv3_b'][cidx]

        # additive map frames: 18 partitions = 2 halves x 9 tap shifts
        a = a32[b].astype(np.float32).reshape(32, 32)
        aup = a[np.arange(192) // 6][:, np.arange(192) // 6]  # (192,192)
        afr = np.zeros((2, FF), np.float32)
        fr0 = np.zeros((FR, FC), np.float32)
        fr0[1:97, 2:194] = aup[0:96]
        fr0[97, 2:194] = aup[96]
        afr[0] = fr0.reshape(FF)
        fr1 = np.zeros((FR, FC), np.float32)
        fr1[1:97, 2:194] = aup[96:192]
        fr1[0, 2:194] = aup[95]
        afr[1] = fr1.reshape(FF)
        af = np.zeros((18, FF), np.float32)
        for h in range(2):
            for t in range(9):
                dlt = _delta(t)
                src = afr[h]
                dst = np.zeros(FF, np.float32)
                if dlt >= 0:
                    dst[:FF - dlt] = src[dlt:]
                else:
                    dst[-dlt:] = src[:FF + dlt]
                af[h * 9 + t] = dst
        maps.append(dict(
            xb=_pad_frame(x[b], BF16),
            x8=_pad_frame(x[b], F8),
            wp8=np.ascontiguousarray(wp8).astype(F8),
            wpb=np.ascontiguousarray(wpb),
            wa=np.ascontiguousarray(wa),
            af=np.ascontiguousarray(af).astype(F8),
            bias=bias))
    return maps


def _diag128(v):
    out = np.zeros((128, 128), np.float32)
    out[np.arange(128), np.arange(128)] = v
    return out


def kernel(**inputs):
    from concourse.bass_utils import run_bass_kernel_spmd

    x = np.asarray(inputs['x'], np.float32)
    d = np.asarray(inputs['d'], np.float32)
    in_maps = _host_precompute(x, d, inputs)

    if 'nc' not in _CACHE:
        _CACHE['nc'] = _build_nc()
    nc = _CACHE['nc']

    try:
        res = run_bass_kernel_spmd(nc, in_maps, list(range(B)))
    except Exception:
        # transient NRT_EXEC_UNIT_UNRECOVERABLE observed on back-to-back
        # runs; a single retry is free and often clears it
        res = run_bass_kernel_spmd(nc, in_maps, list(range(B)))
    out = np.stack([np.asarray(res.results[i]['y'], np.float32).reshape(C, H, W)
                    for i in range(B)])
    return out


# revision 12
# speedup vs baseline: 1.3708x; 1.1974x over previous
"""Trainium2 Bass kernel for the dynamic-attention-block CNN (nn_DAB).

Data-parallel over batch: 8 samples -> 8 NeuronCores. Each core runs the
full per-sample network with activations resident in SBUF as padded
"frames": 128 partitions = 64 channels x 2 image halves, each half a
98x196 zero-padded row-major frame (rows -1..96 / 95..192 of the 192x192
image, cols -2..193).

Conv structure (all single 128-partition matmuls; the two image halves
ride in one instruction via block-diagonal weights):
  - 3x3 convs and dynamic depthwise convs run in fp8e4m3 DoubleRow mode:
    taps are processed in pairs (lhsT [128,2,128], rhs [128,2,N] with the
    pair dim striding between the two tap offsets), 5 passes per conv.
    Weights are pre-scaled by 16 (convs) / 64 (dw) to dodge fp8
    subnormals; the inverse scale is folded into the engine op that
    drains PSUM.
  - 1x1 convs + channel-attention gates run in bf16: the x*att residual
    is an extra diagonal-matrix matmul accumulating into the same PSUM
    group, so no vector-engine gating pass exists at all.
  - The additive 32x32-upsampled map is folded into conv2 as one fp8
    DoubleRow matmul pass (stride-0 pair, second slot zero): 18
    partitions hold the 9 tap-shifted copies of the upsampled map for
    each half, weights are the channel-summed conv2 taps.

The five stages are software-pipelined ACROSS stage boundaries: stage k
visits tiles in an order rotated by 2(k-1), and the halo rows (the only
cross-half data) are DMA'd as soon as their source tiles (47 and 0)
drain, so the next stage's convs never wait on the previous stage's
tail. Engine balance per tile: PE does all matmuls; DVE drains the two
dw-conv PSUMs (prelu, scale folded into the following 1x1 weights) and
conv2's PSUM; Act drains the two da outputs, conv1 and conv3; GPSIMD
makes conv2's fp8 frame copy.
"""

import sys

for _p in ("/opt/trn_rl_repo", "/root/.axon_site/_ro/pypackages"):
    if _p not in sys.path:
        sys.path.insert(0, _p)

import numpy as np
import ml_dtypes

BF16 = ml_dtypes.bfloat16
F8 = ml_dtypes.float8_e4m3

B, C, H, W = 8, 64, 192, 192
HW = H * W
FR, FC = 98, 196          # frame rows / cols per half
FF = FR * FC              # frame elems per partition
ALPHA = 0.1               # leaky slope
WS = 16.0                 # fp8 conv weight pre-scale
KS = 64.0                 # fp8 dw kernel pre-scale

# bias pack columns (the *W columns are pre-scaled by WS for DVE drains,
# which add the bias before the 1/WS multiply; Act drains scale first)
BI_B1, BI_B1W, BI_B2, BI_B2W, BI_B3, BI_B3W, BI_CB1, BI_CB2 = range(8)

# DoubleRow tap pairing: (tap_a, tap_b) with taps t = 3*dy + dx,
# delta(t) = (dy-1)*FC + (dx-1).  5 passes cover all 9 taps; the last
# pass's second slot has zero weight (stride 0 keeps the read in-bounds).
PAIRS = [(0, 1), (3, 4), (6, 7), (2, 5), (8, None)]

# packed fp8 weight slots in wpack8
WP_W1, WP_W2, WP_W3, WP_KD1, WP_KD2 = range(5)
# packed bf16 weight slots in wpackb
WB_CW1, WB_CW2, WB_G1, WB_G2, WB_RID = range(5)

NTILE = 48
ROT = 2                   # per-stage tile-order rotation

_CACHE = {}


def _delta(t):
    return (t // 3 - 1) * FC + (t % 3 - 1)


def _build_nc():
    import concourse.bacc as bacc
    import concourse.mybir as mybir
    from concourse import tile

    f32 = mybir.dt.float32
    bf16 = mybir.dt.bfloat16
    f8 = mybir.dt.float8e4
    AF = mybir.ActivationFunctionType
    ALU = mybir.AluOpType
    DR = mybir.MatmulPerfMode.DoubleRow

    nc = bacc.Bacc("TRN2", target_bir_lowering=False, debug=False, num_devices=8)

    xb_d = nc.dram_tensor("xb", [128, FF], bf16, kind="ExternalInput").ap()
    x8_d = nc.dram_tensor("x8", [128, FF], f8, kind="ExternalInput").ap()
    wp8_d = nc.dram_tensor("wp8", [128, 25, 2, 128], f8, kind="ExternalInput").ap()
    wpb_d = nc.dram_tensor("wpb", [128, 5, 128], bf16, kind="ExternalInput").ap()
    wa_d = nc.dram_tensor("wa", [18, 2, 128], f8, kind="ExternalInput").ap()
    af_d = nc.dram_tensor("af", [18, FF], f8, kind="ExternalInput").ap()
    bias_d = nc.dram_tensor("bias", [128, 8], f32, kind="ExternalInput").ap()
    y_d = nc.dram_tensor("y", [C, HW], f32, kind="ExternalOutput").ap()

    # row-pair sweep tiles: 48 tiles of 2 image rows (392 frame cols)
    qt = [((2 * i + 1) * FC + 2, 2 * FC) for i in range(NTILE)]

    from contextlib import ExitStack
    with tile.TileContext(nc) as tc, ExitStack() as ctx:
        wpool = ctx.enter_context(tc.tile_pool(name="w", bufs=1))
        fbpool = ctx.enter_context(tc.tile_pool(name="fb", bufs=2))
        f8pool = ctx.enter_context(tc.tile_pool(name="f8", bufs=3))
        t1p = ctx.enter_context(tc.tile_pool(name="t1", bufs=6))
        outp = ctx.enter_context(tc.tile_pool(name="outp", bufs=4))
        psA = ctx.enter_context(tc.tile_pool(name="psA", bufs=4, space="PSUM"))
        psB = ctx.enter_context(tc.tile_pool(name="psB", bufs=4, space="PSUM"))

        # ---- weights / constants to SBUF (batched DMAs) ----
        wp8 = wpool.tile([128, 25, 2, 128], f8, tag="wp8")
        wpb = wpool.tile([128, 5, 128], bf16, tag="wpb")
        wa = wpool.tile([18, 2, 128], f8, tag="wa")
        af = wpool.tile([18, FF], f8, tag="af")
        bias = wpool.tile([128, 8], f32, tag="bias")
        nc.sync.dma_start(out=wp8[...], in_=wp8_d)
        nc.gpsimd.dma_start(out=wpb[...], in_=wpb_d)
        nc.gpsimd.dma_start(out=wa[...], in_=wa_d)
        nc.gpsimd.dma_start(out=bias[...], in_=bias_d)
        nc.scalar.dma_start(out=af[...], in_=af_d)

        def w8(slot):
            return wp8[:, 5 * slot:5 * slot + 5, :, :]

        def wb(slot):
            return wpb[:, slot, :]

        def cold(col):
            return bias[:, col:col + 1]

        # ---- input frames (host pre-padded); fp8 frame first ----
        Xb = fbpool.tile([128, FF], bf16, tag="fb")
        X8 = f8pool.tile([128, FF], f8, tag="f8")
        O1 = f8pool.tile([128, FF], f8, tag="f8")
        O2 = f8pool.tile([128, FF], f8, tag="f8")
        O3b = fbpool.tile([128, FF], bf16, tag="fb")
        qs = (nc.sync, nc.scalar, nc.gpsimd)
        step8 = (FF + 2) // 3
        for k in range(3):
            c0, c1 = k * step8, min((k + 1) * step8, FF)
            qs[k].dma_start(out=X8[:, c0:c1], in_=x8_d[:, c0:c1])
        nchunk = 6
        step = (FF + nchunk - 1) // nchunk
        for k in range(nchunk):
            c0, c1 = k * step, min((k + 1) * step, FF)
            qs[k % 3].dma_start(out=Xb[:, c0:c1], in_=xb_d[:, c0:c1])

        # ---- PE warmup: throwaway matmuls while the input DMAs stream
        # in; keeps the p-state ramp finished before real work ----
        wrm = wpool.tile([128, 512], bf16, tag="wrm")
        nc.vector.memset(wrm[:, :], 0.0)
        pw = psA.tile([128, 512], f32, tag="psA")
        for _ in range(14):
            nc.tensor.matmul(pw[:, :], wrm[:, 0:128], wrm[:, :],
                             start=True, stop=True, skip_group_check=True)

        def v3(m):
            return m[:, :].rearrange("p (a b) -> p a b", b=FC)

        # one-time pad zeroing for frame buffers not filled by host DMA.
        # Interior writes never touch pads again, so pads stay zero across
        # all later reuses of these pool buffers.
        for m in (O1, O2, O3b):
            mv = v3(m)
            nc.gpsimd.memset(mv[0:64, 0, :], 0.0)
            nc.gpsimd.memset(mv[64:128, FR - 1, :], 0.0)
            nc.gpsimd.memset(mv[:, :, 0:2], 0.0)
            nc.gpsimd.memset(mv[:, :, FC - 2:FC], 0.0)

        def halo_a(m):
            # half1 top halo row (img 95) <- half0 frame row 96, src tile 47
            mv = v3(m)
            nc.gpsimd.dma_start(out=mv[64:128, 0, :], in_=mv[0:64, 96, :])

        def halo_b(m):
            # half0 bottom halo row (img 96) <- half1 frame row 1, src tile 0
            mv = v3(m)
            nc.gpsimd.dma_start(out=mv[0:64, FR - 1, :], in_=mv[64:128, 1, :])

        def maybe_halo(t, frames):
            if t == 47:
                for m in frames:
                    halo_a(m)
            elif t == 0:
                for m in frames:
                    halo_b(m)

        def order(stage):
            s = (ROT * stage) % NTILE
            return [(s + i) % NTILE for i in range(NTILE)]

        def dr_rhs(m8, q, n, pair):
            ta, tb = pair
            base = q + _delta(ta)
            stride = 0 if tb is None else _delta(tb) - _delta(ta)
            n = min(n, FF - base - max(stride, 0))
            r = m8[:, base:base + 1].copy()
            r.ap[1] = [stride, 2]
            r.ap.append([1, n])
            return r, n

        def conv_dr(ps, wsb, m8, q, n, stop=True):
            # P0 (top-left taps) never clamps, so it is the start pass and
            # always covers the full tile; clamped later passes only lose
            # tail columns that are pad positions, never emitted.
            for p in range(5):
                rhs, np_ = dr_rhs(m8, q, n, PAIRS[p])
                nc.tensor.matmul(ps[:, :np_], wsb[:, p, :, :], rhs,
                                 start=(p == 0), stop=(stop and p == 4),
                                 perf_mode=DR, skip_group_check=True)

        def iview(dst, q):
            # interior-only view: rows of the pair, cols 2:194
            r = q // FC
            return v3(dst)[:, r:r + 2, 2:194]

        def pview(src_ps, n):
            return src_ps[:, :n].rearrange("p (a b) -> p a b", b=FC)[:, :, 0:192]

        # Per-stage drain-engine alternation: each stage's per-engine drain
        # rate must stay below the PE rate or the drain queue backlog
        # throttles PSUM-buffer reuse (and the next stage's drains behind
        # it in the same queue).  Act drains are 1 op (~511ns); DVE prelu
        # drains are 2 ops (~800ns) since scalar_tensor_tensor can't read
        # PSUM, but plain identity+bias drains are 1 DVE op (~533ns).

        def prelu_drain_dve(dst, ps, q, n, cb_col):
            # (psum + cb) on DVE (bf16 staging), then SBUF-only prelu
            tm = t1p.tile([128, 2 * FC], bf16, tag="t1")
            tv = tm[:, :n].rearrange("p (a b) -> p a b", b=FC)[:, :, 0:192]
            nc.vector.tensor_scalar(tv, pview(ps, n), cold(cb_col), None,
                                    op0=ALU.add)
            nc.vector.scalar_tensor_tensor(iview(dst, q), tv, ALPHA, tv,
                                           op0=ALU.mult, op1=ALU.max)

        def prelu_drain_dve_ws(dst, ps, q, n, bw_col):
            # ((psum + WS*b) * 1/WS) on DVE, then SBUF-only prelu
            tm = t1p.tile([128, 2 * FC], bf16, tag="t1")
            tv = tm[:, :n].rearrange("p (a b) -> p a b", b=FC)[:, :, 0:192]
            nc.vector.tensor_scalar(tv, pview(ps, n), cold(bw_col), 1.0 / WS,
                                    op0=ALU.add, op1=ALU.mult)
            nc.vector.scalar_tensor_tensor(iview(dst, q), tv, ALPHA, tv,
                                           op0=ALU.mult, op1=ALU.max)

        def da_stage(stage, inb, in8, kd_slot, cw_slot, g_slot, cb_col, out8):
            # software-pipelined by two tiles: PE issues dw(j), dw(j+1)
            # before the 1x1+gate of tile j-1 so the in-order PE queue
            # rides out the psA->Act t1->cw dependency chain (~1us).
            kd, cw, g = w8(kd_slot), wb(cw_slot), wb(g_slot)

            def tail(prev, i):
                t1, q, n, t = prev
                pb = psB.tile([128, 2 * FC], f32, tag="psB")
                nc.tensor.matmul(pb[:, :n], cw, t1[:, :n],
                                 start=True, stop=False, skip_group_check=True)
                nc.tensor.matmul(pb[:, :n], g, inb[:, q:q + n],
                                 start=False, stop=True, skip_group_check=True)
                if i % 4 == 0:
                    nc.scalar.activation(iview(out8, q), pview(pb, n),
                                         AF.Prelu, bias=cold(cb_col),
                                         alpha=ALPHA)
                else:
                    prelu_drain_dve(out8, pb, q, n, cb_col)
                maybe_halo(t, (out8,))

            pipe = []
            for i, t in enumerate(order(stage)):
                q, n = qt[t]
                pa = psA.tile([128, 2 * FC], f32, tag="psA")
                conv_dr(pa, kd, in8, q, n)
                if len(pipe) == 2:
                    tail(pipe.pop(0), i)
                # t1 = prelu(psA); the KS dw-weight scale rides along
                # (prelu is positively homogeneous) and is divided out of
                # the 1x1 weights on the host.
                t1 = t1p.tile([128, 2 * FC], bf16, tag="t1")
                nc.scalar.activation(t1[:, :n], pa[:, :n], AF.Prelu,
                                     alpha=ALPHA)
                pipe.append((t1, q, n, t))
            for i, prev in enumerate(pipe):
                tail(prev, NTILE + i)

        # ---- network ----
        da_stage(0, Xb, X8, WP_KD1, WB_CW1, WB_G1, BI_CB1, O1)

        # conv1 -> prelu -> fp8 frame
        for i, t in enumerate(order(1)):
            q, n = qt[t]
            pa = psA.tile([128, 2 * FC], f32, tag="psA")
            conv_dr(pa, w8(WP_W1), O1, q, n)
            if i % 5 < 3:
                nc.scalar.activation(iview(O2, q), pview(pa, n), AF.Prelu,
                                     scale=1.0 / WS, bias=cold(BI_B1),
                                     alpha=ALPHA)
            else:
                prelu_drain_dve_ws(O2, pa, q, n, BI_B1W)
            maybe_halo(t, (O2,))

        # conv2 (+ additive map as a stride-0 fp8 DR pass) -> identity+bias
        # -> bf16 frame + fp8 copy
        O38 = f8pool.tile([128, FF], f8, tag="f8")
        for i, t in enumerate(order(2)):
            q, n = qt[t]
            pa = psA.tile([128, 2 * FC], f32, tag="psA")
            conv_dr(pa, w8(WP_W2), O2, q, n, stop=False)
            r = af[:, q:q + 1].copy()
            r.ap[1] = [0, 2]
            r.ap.append([1, n])
            nc.tensor.matmul(pa[:, :n], wa[:, :, :], r,
                             start=False, stop=True, perf_mode=DR,
                             skip_group_check=True)
            if i % 2 == 0:
                nc.scalar.activation(iview(O3b, q), pview(pa, n), AF.Identity,
                                     scale=1.0 / WS, bias=cold(BI_B2))
                nc.gpsimd.tensor_copy(O38[:, q:q + n], O3b[:, q:q + n])
            else:
                nc.vector.tensor_scalar(iview(O3b, q), pview(pa, n),
                                        cold(BI_B2W), 1.0 / WS,
                                        op0=ALU.add, op1=ALU.mult)
                nc.vector.tensor_copy(O38[:, q:q + n], O3b[:, q:q + n])
            maybe_halo(t, (O38,))

        O4 = f8pool.tile([128, FF], f8, tag="f8")
        da_stage(3, O3b, O38, WP_KD2, WB_CW2, WB_G2, BI_CB2, O4)

        # ---- conv3 + residual: x (bf16, scaled by WS via diag weights)
        # and WS*b3 accumulate straight into PSUM; drain with 1/WS ----
        for j, t in enumerate(order(4)):
            q, n = qt[t]
            pa = psA.tile([128, 2 * FC], f32, tag="psA")
            conv_dr(pa, w8(WP_W3), O4, q, n, stop=False)
            nc.tensor.matmul(pa[:, :n], wb(WB_RID), Xb[:, q:q + n],
                             start=False, stop=True, skip_group_check=True)
            ot = outp.tile([128, 2, 192], f32, tag="ot")
            if j % 2 == 0:
                nc.scalar.activation(ot[:, :, :], pview(pa, n), AF.Identity,
                                     scale=1.0 / WS, bias=cold(BI_B3))
            else:
                nc.vector.tensor_scalar(ot[:, :, :], pview(pa, n),
                                        cold(BI_B3W), 1.0 / WS,
                                        op0=ALU.add, op1=ALU.mult)
            r0 = q // FC - 1  # image row of the pair
            qs[j % 3].dma_start(
                out=y_d[:, r0 * 192:(r0 + 2) * 192]
                .rearrange("p (r c) -> p r c", c=192),
                in_=ot[0:64, :, :])
            qs[(j + 1) % 3].dma_start(
                out=y_d[:, (96 + r0) * 192:(96 + r0 + 2) * 192]
                .rearrange("p (r c) -> p r c", c=192),
                in_=ot[64:128, :, :])

    nc.compile()
    return nc


def _pad_frame(xb, dtype):
    """(64,192,192) fp32 -> (128, FR*FC) dual-half padded frame."""
    fr = np.zeros((128, FR, FC), np.float32)
    fr[0:64, 1:97, 2:194] = xb[:, 0:96, :]
    fr[0:64, 97, 2:194] = xb[:, 96, :]
    fr[64:128, 1:97, 2:194] = xb[:, 96:192, :]
    fr[64:128, 0, 2:194] = xb[:, 95, :]
    return np.ascontiguousarray(fr.reshape(128, FF)).astype(dtype)


def _leaky_np(v):
    return np.where(v >= 0, v, ALPHA * v)


def _host_precompute(x, d, p):
    """Build per-core input maps. p: dict of raw weight arrays."""
    d = d.astype(np.float64)
    kern = {}
    att = {}
    for i in (1, 2):
        kw1, kw2 = p[f'da{i}_kw1'].astype(np.float64), p[f'da{i}_kw2'].astype(np.float64)
        ca1, ca2 = p[f'da{i}_ca1'].astype(np.float64), p[f'da{i}_ca2'].astype(np.float64)
        kern[i] = _leaky_np(d @ kw1.T) @ kw2.T          # (B, 576) [c*9+t]
        z = _leaky_np(d @ ca1.T) @ ca2.T
        att[i] = 1.0 / (1.0 + np.exp(-z))               # (B, 64)
    a32 = _leaky_np(d @ p['add_w1'].astype(np.float64).T) @ \
        p['add_w2'].astype(np.float64).T                # (B, 1024)

    cidx = np.arange(128) % 64

    def convw_dr(w):
        # (O, C, 3, 3) fp32 -> [128, 5, 2, 128] f8 block-diag DoubleRow taps
        wq = (w.astype(np.float32) * WS).astype(F8).astype(np.float32)
        wt = wq.transpose(1, 2, 3, 0).reshape(64, 9, 64)  # [c, t, o]
        out = np.zeros((128, 5, 2, 128), np.float32)
        for pi, (ta, tb) in enumerate(PAIRS):
            blk = np.zeros((64, 2, 64), np.float32)
            blk[:, 0, :] = wt[:, ta, :]
            if tb is not None:
                blk[:, 1, :] = wt[:, tb, :]
            out[0:64, pi, :, 0:64] = blk
            out[64:128, pi, :, 64:128] = blk
        return out.astype(F8)

    def cw_bd(w, scale=1.0):
        # (O, C) -> [128, 128] block-diag: [p, o]
        out = np.zeros((128, 128), np.float32)
        out[0:64, 0:64] = w.T * scale
        out[64:128, 64:128] = w.T * scale
        return out

    # fp8 packed conv/dw weights (per-sample kd slots filled below)
    w1 = convw_dr(p['conv1_w'])
    w2 = convw_dr(p['conv2_w'])
    w3 = convw_dr(p['conv3_w'])
    # 1/KS folds the dw pre-scale out of the un-scaled DVE t1 prelu
    cw1 = cw_bd(p['da1_cw'], 1.0 / KS)
    cw2 = cw_bd(p['da2_cw'], 1.0 / KS)

    # additive-map conv weights: wa[(h,t), 0, o_col] = WS * sum_c w2[o,c,t]
    w2sum = p['conv2_w'].astype(np.float64).sum(axis=1).reshape(64, 9)  # [o, t]
    wa = np.zeros((18, 2, 128), np.float32)
    for h in range(2):
        for t in range(9):
            wa[h * 9 + t, 0, h * 64:(h + 1) * 64] = WS * w2sum[:, t]
    wa = wa.astype(F8)

    rid = _diag128(np.full(128, WS, np.float32))

    maps = []
    for b in range(B):
        kd = {}
        for i in (1, 2):
            kc = (kern[i][b].reshape(64, 9).astype(np.float32) * KS) \
                .astype(F8).astype(np.float32)           # [c, t]
            kdl = np.zeros((128, 5, 2, 128), np.float32)
            for pi, (ta, tb) in enumerate(PAIRS):
                kdl[np.arange(128), pi, 0, np.arange(128)] = kc[cidx, ta]
                if tb is not None:
                    kdl[np.arange(128), pi, 1, np.arange(128)] = kc[cidx, tb]
            kd[i] = kdl.astype(F8)
        g = {i: _diag128(att[i][b][cidx]) for i in (1, 2)}
        wp8 = np.concatenate(
            [w1, w2, w3, kd[1], kd[2]], axis=1).reshape(128, 25, 2, 128)
        wpb = np.stack(
            [cw1, cw2, g[1], g[2], rid], axis=1).astype(BF16)
        bias = np.zeros((128, 8), np.float32)
        bias[:, BI_B1] = p['conv1_b'][cidx]
        bias[:, BI_B1W] = WS * p['conv1_b'][cidx]
        bias[:, BI_B2] = p['conv2_b'][cidx]
        bias[:, BI_B2W] = WS * p['conv2_b'][cidx]
        bias[:, BI_B3] = p['conv3_b'][cidx]
        bias[:, BI_B3W] = WS * p['conv3_b'][cidx]
        bias[:, BI_CB1] = p['da1_cb'][cidx]
        bias[:, BI_CB2] = p['da2_cb'][cidx]

        # additive map frames: 18 partitions = 2 halves x 9 tap shifts
        a = a32[b].astype(np.float32).reshape(32, 32)
        aup = a[np.arange(192) // 6][:, np.arange(192) // 6]  # (192,192)
        afr = np.zeros((2, FF), np.float32)
        fr0 = np.zeros((FR, FC), np.float32)
        fr0[1:97, 2:194] = aup[0:96]
        fr0[97, 2:194] = aup[96]
        afr[0] = fr0.reshape(FF)
        fr1 = np.zeros((FR, FC), np.float32)
        fr1[1:97, 2:194] = aup[96:192]
        fr1[0, 2:194] = aup[95]
        afr[1] = fr1.reshape(FF)
        af = np.zeros((18, FF), np.float32)
        for h in range(2):
            for t in range(9):
                dlt = _delta(t)
                src = afr[h]
                dst = np.zeros(FF, np.float32)
                if dlt >= 0:
                    dst[:FF - dlt] = src[dlt:]
                else:
                    dst[-dlt:] = src[:FF + dlt]
                af[h * 9 + t] = dst
        maps.append(dict(
            xb=_pad_frame(x[b], BF16),
            x8=_pad_frame(x[b], F8),
            wp8=np.ascontiguousarray(wp8).astype(F8),
            wpb=np.ascontiguousarray(wpb),
            wa=np.ascontiguousarray(wa),
            af=np.ascontiguousarray(af).astype(F8),
            bias=bias))
    return maps


def _diag128(v):
    out = np.zeros((128, 128), np.float32)
    out[np.arange(128), np.arange(128)] = v
    return out


def kernel(**inputs):
    from concourse.bass_utils import run_bass_kernel_spmd

    x = np.asarray(inputs['x'], np.float32)
    d = np.asarray(inputs['d'], np.float32)
    in_maps = _host_precompute(x, d, inputs)

    if 'nc' not in _CACHE:
        _CACHE['nc'] = _build_nc()
    nc = _CACHE['nc']

    try:
        res = run_bass_kernel_spmd(nc, in_maps, list(range(B)))
    except Exception:
        # transient NRT_EXEC_UNIT_UNRECOVERABLE observed on back-to-back
        # runs; a single retry is free and often clears it
        res = run_bass_kernel_spmd(nc, in_maps, list(range(B)))
    out = np.stack([np.asarray(res.results[i]['y'], np.float32).reshape(C, H, W)
                    for i in range(B)])
    return out


# revision 14
# speedup vs baseline: 1.5134x; 1.1040x over previous
"""Trainium2 Bass kernel for the dynamic-attention-block CNN (nn_DAB).

Data-parallel over batch: 8 samples -> 8 NeuronCores. Each core runs the
full per-sample network with activations resident in SBUF as padded
"frames": 128 partitions = 64 channels x 2 image halves, each half a
98x196 zero-padded row-major frame (rows -1..96 / 95..192 of the 192x192
image, cols -2..193).

Conv structure (all single 128-partition matmuls; the two image halves
ride in one instruction via block-diagonal weights):
  - 3x3 convs and dynamic depthwise convs run in fp8e4m3 DoubleRow mode:
    taps are processed in pairs (lhsT [128,2,128], rhs [128,2,N] with the
    pair dim striding between the two tap offsets), 5 passes per conv.
    Weights are pre-scaled by 16 (convs) / 64 (dw) to dodge fp8
    subnormals; the inverse scale is folded into the engine op that
    drains PSUM.
  - 1x1 convs + channel-attention gates run in bf16: the x*att residual
    is an extra diagonal-matrix matmul accumulating into the same PSUM
    group, so no vector-engine gating pass exists at all.
  - The additive 32x32-upsampled map is folded into conv2 as one fp8
    DoubleRow matmul pass (stride-0 pair, second slot zero): 18
    partitions hold the 9 tap-shifted copies of the upsampled map for
    each half, weights are the channel-summed conv2 taps.

The five stages are software-pipelined ACROSS stage boundaries: stage k
visits tiles in an order rotated by 2(k-1), and the halo rows (the only
cross-half data) are DMA'd as soon as their source tiles (47 and 0)
drain, so the next stage's convs never wait on the previous stage's
tail. Engine balance per tile: PE does all matmuls; DVE drains the two
dw-conv PSUMs (prelu, scale folded into the following 1x1 weights) and
conv2's PSUM; Act drains the two da outputs, conv1 and conv3; GPSIMD
makes conv2's fp8 frame copy.
"""

import sys

for _p in ("/opt/trn_rl_repo", "/root/.axon_site/_ro/pypackages"):
    if _p not in sys.path:
        sys.path.insert(0, _p)

import numpy as np
import ml_dtypes

BF16 = ml_dtypes.bfloat16
F8 = ml_dtypes.float8_e4m3

B, C, H, W = 8, 64, 192, 192
HW = H * W
FR, FC = 98, 196          # frame rows / cols per half
FF = FR * FC              # frame elems per partition
ALPHA = 0.1               # leaky slope
WS = 16.0                 # fp8 conv weight pre-scale
KS = 64.0                 # fp8 dw kernel pre-scale

# bias pack columns (the *W columns are pre-scaled by WS for DVE drains,
# which add the bias before the 1/WS multiply; Act drains scale first)
BI_B1, BI_B1W, BI_B2, BI_B2W, BI_B3, BI_B3W, BI_CB1, BI_CB2 = range(8)

# DoubleRow tap pairing: (tap_a, tap_b) with taps t = 3*dy + dx,
# delta(t) = (dy-1)*FC + (dx-1).  5 passes cover all 9 taps; the last
# pass's second slot has zero weight (stride 0 keeps the read in-bounds).
PAIRS = [(0, 1), (3, 4), (6, 7), (2, 5), (8, None)]

# packed fp8 weight slots in wpack8
WP_W1, WP_W2, WP_W3, WP_KD1, WP_KD2 = range(5)
# packed bf16 weight slots in wpackb
WB_CW1, WB_CW2, WB_G1, WB_G2, WB_RID = range(5)

NTILE = 48
ROT = 2                   # per-stage tile-order rotation

_CACHE = {}


def _delta(t):
    return (t // 3 - 1) * FC + (t % 3 - 1)


def _build_nc():
    import concourse.bacc as bacc
    import concourse.mybir as mybir
    from concourse import tile

    f32 = mybir.dt.float32
    bf16 = mybir.dt.bfloat16
    f8 = mybir.dt.float8e4
    AF = mybir.ActivationFunctionType
    ALU = mybir.AluOpType
    DR = mybir.MatmulPerfMode.DoubleRow

    nc = bacc.Bacc("TRN2", target_bir_lowering=False, debug=False, num_devices=8)

    xb_d = nc.dram_tensor("xb", [128, FF], bf16, kind="ExternalInput").ap()
    x8_d = nc.dram_tensor("x8", [128, FF], f8, kind="ExternalInput").ap()
    wp8_d = nc.dram_tensor("wp8", [128, 25, 2, 128], f8, kind="ExternalInput").ap()
    wpb_d = nc.dram_tensor("wpb", [128, 5, 128], bf16, kind="ExternalInput").ap()
    wa_d = nc.dram_tensor("wa", [18, 2, 128], f8, kind="ExternalInput").ap()
    af_d = nc.dram_tensor("af", [18, FF], f8, kind="ExternalInput").ap()
    bias_d = nc.dram_tensor("bias", [128, 8], f32, kind="ExternalInput").ap()
    y_d = nc.dram_tensor("y", [C, HW], f32, kind="ExternalOutput").ap()

    # row-pair sweep tiles: 48 tiles of 2 image rows (392 frame cols)
    qt = [((2 * i + 1) * FC + 2, 2 * FC) for i in range(NTILE)]

    from contextlib import ExitStack
    with tile.TileContext(nc) as tc, ExitStack() as ctx:
        wpool = ctx.enter_context(tc.tile_pool(name="w", bufs=1))
        fbpool = ctx.enter_context(tc.tile_pool(name="fb", bufs=2))
        f8pool = ctx.enter_context(tc.tile_pool(name="f8", bufs=3))
        t1p = ctx.enter_context(tc.tile_pool(name="t1", bufs=6))
        outp = ctx.enter_context(tc.tile_pool(name="outp", bufs=4))
        psA = ctx.enter_context(tc.tile_pool(name="psA", bufs=4, space="PSUM"))
        psB = ctx.enter_context(tc.tile_pool(name="psB", bufs=4, space="PSUM"))

        # ---- weights / constants to SBUF (batched DMAs) ----
        wp8 = wpool.tile([128, 25, 2, 128], f8, tag="wp8")
        wpb = wpool.tile([128, 5, 128], bf16, tag="wpb")
        wa = wpool.tile([18, 2, 128], f8, tag="wa")
        af = wpool.tile([18, FF], f8, tag="af")
        bias = wpool.tile([128, 8], f32, tag="bias")
        nc.scalar.dma_start(out=wp8[...], in_=wp8_d)
        nc.gpsimd.dma_start(out=wpb[...], in_=wpb_d)
        nc.gpsimd.dma_start(out=wa[...], in_=wa_d)
        nc.gpsimd.dma_start(out=bias[...], in_=bias_d)
        nc.scalar.dma_start(out=af[...], in_=af_d)

        def w8(slot):
            return wp8[:, 5 * slot:5 * slot + 5, :, :]

        def wb(slot):
            return wpb[:, slot, :]

        def cold(col):
            return bias[:, col:col + 1]

        # ---- input frames (host pre-padded); fp8 frame first ----
        Xb = fbpool.tile([128, FF], bf16, tag="fb")
        X8 = f8pool.tile([128, FF], f8, tag="f8")
        O1 = f8pool.tile([128, FF], f8, tag="f8")
        O2 = f8pool.tile([128, FF], f8, tag="f8")
        O3b = fbpool.tile([128, FF], bf16, tag="fb")
        qs = (nc.sync, nc.scalar, nc.gpsimd)
        step8 = (FF + 2) // 3
        for k in range(3):
            c0, c1 = k * step8, min((k + 1) * step8, FF)
            qs[k].dma_start(out=X8[:, c0:c1], in_=x8_d[:, c0:c1])
        nchunk = 6
        step = (FF + nchunk - 1) // nchunk
        for k in range(nchunk):
            c0, c1 = k * step, min((k + 1) * step, FF)
            qs[k % 3].dma_start(out=Xb[:, c0:c1], in_=xb_d[:, c0:c1])

        # ---- PE warmup: throwaway matmuls while the input DMAs stream
        # in; keeps the p-state ramp finished before real work ----
        wrm = wpool.tile([128, 512], bf16, tag="wrm")
        nc.vector.memset(wrm[:, :], 0.0)
        pw = psA.tile([128, 512], f32, tag="psA")
        for _ in range(14):
            nc.tensor.matmul(pw[:, :], wrm[:, 0:128], wrm[:, :],
                             start=True, stop=True, skip_group_check=True)

        def v3(m):
            return m[:, :].rearrange("p (a b) -> p a b", b=FC)

        # one-time pad zeroing for frame buffers not filled by host DMA.
        # Interior writes never touch pads again, so pads stay zero across
        # all later reuses of these pool buffers.
        for m in (O1, O2, O3b):
            mv = v3(m)
            nc.gpsimd.memset(mv[0:64, 0, :], 0.0)
            nc.gpsimd.memset(mv[64:128, FR - 1, :], 0.0)
            nc.gpsimd.memset(mv[:, :, 0:2], 0.0)
            nc.gpsimd.memset(mv[:, :, FC - 2:FC], 0.0)

        def halo_a(m):
            # half1 top halo row (img 95) <- half0 frame row 96, src tile 47
            mv = v3(m)
            nc.gpsimd.dma_start(out=mv[64:128, 0, :], in_=mv[0:64, 96, :])

        def halo_b(m):
            # half0 bottom halo row (img 96) <- half1 frame row 1, src tile 0
            mv = v3(m)
            nc.gpsimd.dma_start(out=mv[0:64, FR - 1, :], in_=mv[64:128, 1, :])

        def maybe_halo(t, frames):
            if t == 47:
                for m in frames:
                    halo_a(m)
            elif t == 0:
                for m in frames:
                    halo_b(m)

        def order(stage):
            s = (ROT * stage) % NTILE
            return [(s + i) % NTILE for i in range(NTILE)]

        def dr_rhs(m8, q, n, pair):
            ta, tb = pair
            base = q + _delta(ta)
            stride = 0 if tb is None else _delta(tb) - _delta(ta)
            n = min(n, FF - base - max(stride, 0))
            r = m8[:, base:base + 1].copy()
            r.ap[1] = [stride, 2]
            r.ap.append([1, n])
            return r, n

        def conv_dr(ps, wsb, m8, q, n, stop=True):
            # P0 (top-left taps) never clamps, so it is the start pass and
            # always covers the full tile; clamped later passes only lose
            # tail columns that are pad positions, never emitted.
            for p in range(5):
                rhs, np_ = dr_rhs(m8, q, n, PAIRS[p])
                nc.tensor.matmul(ps[:, :np_], wsb[:, p, :, :], rhs,
                                 start=(p == 0), stop=(stop and p == 4),
                                 perf_mode=DR, skip_group_check=True)

        def iview(dst, q):
            # interior-only view: rows of the pair, cols 2:194
            r = q // FC
            return v3(dst)[:, r:r + 2, 2:194]

        def pview(src_ps, n):
            return src_ps[:, :n].rearrange("p (a b) -> p a b", b=FC)[:, :, 0:192]

        # Per-stage drain-engine alternation: each stage's per-engine drain
        # rate must stay below the PE rate or the drain queue backlog
        # throttles PSUM-buffer reuse (and the next stage's drains behind
        # it in the same queue).  Act drains are 1 op (~511ns); DVE prelu
        # drains are 2 ops (~800ns) since scalar_tensor_tensor can't read
        # PSUM, but plain identity+bias drains are 1 DVE op (~533ns).

        def prelu_drain_dve(dst, ps, q, n, cb_col):
            # (psum + cb) on DVE (bf16 staging), then SBUF-only prelu
            tm = t1p.tile([128, 2 * FC], bf16, tag="t1")
            tv = tm[:, :n].rearrange("p (a b) -> p a b", b=FC)[:, :, 0:192]
            nc.vector.tensor_scalar(tv, pview(ps, n), cold(cb_col), None,
                                    op0=ALU.add)
            nc.vector.scalar_tensor_tensor(iview(dst, q), tv, ALPHA, tv,
                                           op0=ALU.mult, op1=ALU.max)

        def prelu_drain_dve_ws(dst, ps, q, n, bw_col):
            # ((psum + WS*b) * 1/WS) on DVE, then SBUF-only prelu
            tm = t1p.tile([128, 2 * FC], bf16, tag="t1")
            tv = tm[:, :n].rearrange("p (a b) -> p a b", b=FC)[:, :, 0:192]
            nc.vector.tensor_scalar(tv, pview(ps, n), cold(bw_col), 1.0 / WS,
                                    op0=ALU.add, op1=ALU.mult)
            nc.vector.scalar_tensor_tensor(iview(dst, q), tv, ALPHA, tv,
                                           op0=ALU.mult, op1=ALU.max)

        def da_stage(stage, inb, in8, kd_slot, cw_slot, g_slot, cb_col, out8):
            # software-pipelined by two tiles: PE issues dw(j), dw(j+1)
            # before the 1x1+gate of tile j-1 so the in-order PE queue
            # rides out the psA->Act t1->cw dependency chain (~1us).
            kd, cw, g = w8(kd_slot), wb(cw_slot), wb(g_slot)

            def tail(prev, i):
                t1, q, n, t = prev
                pb = psB.tile([128, 2 * FC], f32, tag="psB")
                nc.tensor.matmul(pb[:, :n], cw, t1[:, :n],
                                 start=True, stop=False, skip_group_check=True)
                nc.tensor.matmul(pb[:, :n], g, inb[:, q:q + n],
                                 start=False, stop=True, skip_group_check=True)
                if i % 4 == 0:
                    nc.scalar.activation(iview(out8, q), pview(pb, n),
                                         AF.Prelu, bias=cold(cb_col),
                                         alpha=ALPHA)
                else:
                    prelu_drain_dve(out8, pb, q, n, cb_col)
                maybe_halo(t, (out8,))

            pipe = []
            for i, t in enumerate(order(stage)):
                q, n = qt[t]
                pa = psA.tile([128, 2 * FC], f32, tag="psA")
                conv_dr(pa, kd, in8, q, n)
                if len(pipe) == 2:
                    tail(pipe.pop(0), i)
                # t1 = prelu(psA); the KS dw-weight scale rides along
                # (prelu is positively homogeneous) and is divided out of
                # the 1x1 weights on the host.
                t1 = t1p.tile([128, 2 * FC], bf16, tag="t1")
                nc.scalar.activation(t1[:, :n], pa[:, :n], AF.Prelu,
                                     alpha=ALPHA)
                pipe.append((t1, q, n, t))
            for i, prev in enumerate(pipe):
                tail(prev, NTILE + i)

        # ---- network ----
        da_stage(0, Xb, X8, WP_KD1, WB_CW1, WB_G1, BI_CB1, O1)

        # conv1 -> prelu -> fp8 frame
        for i, t in enumerate(order(1)):
            q, n = qt[t]
            pa = psA.tile([128, 2 * FC], f32, tag="psA")
            conv_dr(pa, w8(WP_W1), O1, q, n)
            if i % 5 < 3:
                nc.scalar.activation(iview(O2, q), pview(pa, n), AF.Prelu,
                                     scale=1.0 / WS, bias=cold(BI_B1),
                                     alpha=ALPHA)
            else:
                prelu_drain_dve_ws(O2, pa, q, n, BI_B1W)
            maybe_halo(t, (O2,))

        # conv2 (+ additive map as a stride-0 fp8 DR pass) -> identity+bias
        # -> bf16 frame + fp8 copy
        O38 = f8pool.tile([128, FF], f8, tag="f8")
        for i, t in enumerate(order(2)):
            q, n = qt[t]
            pa = psA.tile([128, 2 * FC], f32, tag="psA")
            conv_dr(pa, w8(WP_W2), O2, q, n, stop=False)
            r = af[:, q:q + 1].copy()
            r.ap[1] = [0, 2]
            r.ap.append([1, n])
            nc.tensor.matmul(pa[:, :n], wa[:, :, :], r,
                             start=False, stop=True, perf_mode=DR,
                             skip_group_check=True)
            if i % 2 == 0:
                nc.scalar.activation(iview(O3b, q), pview(pa, n), AF.Identity,
                                     scale=1.0 / WS, bias=cold(BI_B2))
                nc.gpsimd.tensor_copy(O38[:, q:q + n], O3b[:, q:q + n])
            else:
                nc.vector.tensor_scalar(iview(O3b, q), pview(pa, n),
                                        cold(BI_B2W), 1.0 / WS,
                                        op0=ALU.add, op1=ALU.mult)
                nc.vector.tensor_copy(O38[:, q:q + n], O3b[:, q:q + n])
            maybe_halo(t, (O38,))

        O4 = f8pool.tile([128, FF], f8, tag="f8")
        da_stage(3, O3b, O38, WP_KD2, WB_CW2, WB_G2, BI_CB2, O4)

        # ---- conv3 + residual: x (bf16, scaled by WS via diag weights)
        # and WS*b3 accumulate straight into PSUM; drain with 1/WS.
        # Output rides in 4-tile (8-row) groups so each group is just two
        # DMAs, both on the otherwise-idle sync queue (HWDGE): per-DMA
        # issue costs would otherwise throttle the tail of the pipeline.
        # order(4) starts at tile 8 (4-aligned), so groups of 4 successive
        # positions cover 4 consecutive tiles even across the wrap. ----
        ot = None
        for j, t in enumerate(order(4)):
            q, n = qt[t]
            pa = psA.tile([128, 2 * FC], f32, tag="psA")
            conv_dr(pa, w8(WP_W3), O4, q, n, stop=False)
            nc.tensor.matmul(pa[:, :n], wb(WB_RID), Xb[:, q:q + n],
                             start=False, stop=True, skip_group_check=True)
            k = j % 4
            if k == 0:
                ot = outp.tile([128, 8, 192], f32, tag="ot")
                g0 = q // FC - 1  # image row of the group's first pair
            otv = ot[:, 2 * k:2 * k + 2, :]
            if j % 2 == 0:
                nc.scalar.activation(otv, pview(pa, n), AF.Identity,
                                     scale=1.0 / WS, bias=cold(BI_B3))
            else:
                nc.vector.tensor_scalar(otv, pview(pa, n),
                                        cold(BI_B3W), 1.0 / WS,
                                        op0=ALU.add, op1=ALU.mult)
            if k == 3:
                nc.sync.dma_start(
                    out=y_d[:, g0 * 192:(g0 + 8) * 192]
                    .rearrange("p (r c) -> p r c", c=192),
                    in_=ot[0:64, :, :])
                nc.sync.dma_start(
                    out=y_d[:, (96 + g0) * 192:(96 + g0 + 8) * 192]
                    .rearrange("p (r c) -> p r c", c=192),
                    in_=ot[64:128, :, :])

    nc.compile()
    return nc


def _pad_frame(xb, dtype):
    """(64,192,192) fp32 -> (128, FR*FC) dual-half padded frame."""
    fr = np.zeros((128, FR, FC), np.float32)
    fr[0:64, 1:97, 2:194] = xb[:, 0:96, :]
    fr[0:64, 97, 2:194] = xb[:, 96, :]
    fr[64:128, 1:97, 2:194] = xb[:, 96:192, :]
    fr[64:128, 0, 2:194] = xb[:, 95, :]
    return np.ascontiguousarray(fr.reshape(128, FF)).astype(dtype)


def _leaky_np(v):
    return np.where(v >= 0, v, ALPHA * v)


def _host_precompute(x, d, p):
    """Build per-core input maps. p: dict of raw weight arrays."""
    d = d.astype(np.float64)
    kern = {}
    att = {}
    for i in (1, 2):
        kw1, kw2 = p[f'da{i}_kw1'].astype(np.float64), p[f'da{i}_kw2'].astype(np.float64)
        ca1, ca2 = p[f'da{i}_ca1'].astype(np.float64), p[f'da{i}_ca2'].astype(np.float64)
        kern[i] = _leaky_np(d @ kw1.T) @ kw2.T          # (B, 576) [c*9+t]
        z = _leaky_np(d @ ca1.T) @ ca2.T
        att[i] = 1.0 / (1.0 + np.exp(-z))               # (B, 64)
    a32 = _leaky_np(d @ p['add_w1'].astype(np.float64).T) @ \
        p['add_w2'].astype(np.float64).T                # (B, 1024)

    cidx = np.arange(128) % 64

    def convw_dr(w):
        # (O, C, 3, 3) fp32 -> [128, 5, 2, 128] f8 block-diag DoubleRow taps
        wq = (w.astype(np.float32) * WS).astype(F8).astype(np.float32)
        wt = wq.transpose(1, 2, 3, 0).reshape(64, 9, 64)  # [c, t, o]
        out = np.zeros((128, 5, 2, 128), np.float32)
        for pi, (ta, tb) in enumerate(PAIRS):
            blk = np.zeros((64, 2, 64), np.float32)
            blk[:, 0, :] = wt[:, ta, :]
            if tb is not None:
                blk[:, 1, :] = wt[:, tb, :]
            out[0:64, pi, :, 0:64] = blk
            out[64:128, pi, :, 64:128] = blk
        return out.astype(F8)

    def cw_bd(w, scale=1.0):
        # (O, C) -> [128, 128] block-diag: [p, o]
        out = np.zeros((128, 128), np.float32)
        out[0:64, 0:64] = w.T * scale
        out[64:128, 64:128] = w.T * scale
        return out

    # fp8 packed conv/dw weights (per-sample kd slots filled below)
    w1 = convw_dr(p['conv1_w'])
    w2 = convw_dr(p['conv2_w'])
    w3 = convw_dr(p['conv3_w'])
    # 1/KS folds the dw pre-scale out of the un-scaled DVE t1 prelu
    cw1 = cw_bd(p['da1_cw'], 1.0 / KS)
    cw2 = cw_bd(p['da2_cw'], 1.0 / KS)

    # additive-map conv weights: wa[(h,t), 0, o_col] = WS * sum_c w2[o,c,t]
    w2sum = p['conv2_w'].astype(np.float64).sum(axis=1).reshape(64, 9)  # [o, t]
    wa = np.zeros((18, 2, 128), np.float32)
    for h in range(2):
        for t in range(9):
            wa[h * 9 + t, 0, h * 64:(h + 1) * 64] = WS * w2sum[:, t]
    wa = wa.astype(F8)

    rid = _diag128(np.full(128, WS, np.float32))

    maps = []
    for b in range(B):
        kd = {}
        for i in (1, 2):
            kc = (kern[i][b].reshape(64, 9).astype(np.float32) * KS) \
                .astype(F8).astype(np.float32)           # [c, t]
            kdl = np.zeros((128, 5, 2, 128), np.float32)
            for pi, (ta, tb) in enumerate(PAIRS):
                kdl[np.arange(128), pi, 0, np.arange(128)] = kc[cidx, ta]
                if tb is not None:
                    kdl[np.arange(128), pi, 1, np.arange(128)] = kc[cidx, tb]
            kd[i] = kdl.astype(F8)
        g = {i: _diag128(att[i][b][cidx]) for i in (1, 2)}
        wp8 = np.concatenate(
            [w1, w2, w3, kd[1], kd[2]], axis=1).reshape(128, 25, 2, 128)
        wpb = np.stack(
            [cw1, cw2, g[1], g[2], rid], axis=1).astype(BF16)
        bias = np.zeros((128, 8), np.float32)
        bias[:, BI_B1] = p['conv1_b'][cidx]
        bias[:, BI_B1W] = WS * p['conv1_b'][cidx]
        bias[:, BI_B2] = p['conv2_b'][cidx]
        bias[:, BI_B2W] = WS * p['conv2_b'][cidx]
        bias[:, BI_B3] = p['conv3_b'][cidx]
        bias[:, BI_B3W] = WS * p['conv3_b'][cidx]
        bias[:, BI_CB1] = p['da1_cb'][cidx]
        bias[:, BI_CB2] = p['da2_cb'][cidx]

        # additive map frames: 18 partitions = 2 halves x 9 tap shifts
        a = a32[b].astype(np.float32).reshape(32, 32)
        aup = a[np.arange(192) // 6][:, np.arange(192) // 6]  # (192,192)
        afr = np.zeros((2, FF), np.float32)
        fr0 = np.zeros((FR, FC), np.float32)
        fr0[1:97, 2:194] = aup[0:96]
        fr0[97, 2:194] = aup[96]
        afr[0] = fr0.reshape(FF)
        fr1 = np.zeros((FR, FC), np.float32)
        fr1[1:97, 2:194] = aup[96:192]
        fr1[0, 2:194] = aup[95]
        afr[1] = fr1.reshape(FF)
        af = np.zeros((18, FF), np.float32)
        for h in range(2):
            for t in range(9):
                dlt = _delta(t)
                src = afr[h]
                dst = np.zeros(FF, np.float32)
                if dlt >= 0:
                    dst[:FF - dlt] = src[dlt:]
                else:
                    dst[-dlt:] = src[:FF + dlt]
                af[h * 9 + t] = dst
        maps.append(dict(
            xb=_pad_frame(x[b], BF16),
            x8=_pad_frame(x[b], F8),
            wp8=np.ascontiguousarray(wp8).astype(F8),
            wpb=np.ascontiguousarray(wpb),
            wa=np.ascontiguousarray(wa),
            af=np.ascontiguousarray(af).astype(F8),
            bias=bias))
    return maps


def _diag128(v):
    out = np.zeros((128, 128), np.float32)
    out[np.arange(128), np.arange(128)] = v
    return out


def kernel(**inputs):
    from concourse.bass_utils import run_bass_kernel_spmd

    x = np.asarray(inputs['x'], np.float32)
    d = np.asarray(inputs['d'], np.float32)
    in_maps = _host_precompute(x, d, inputs)

    if 'nc' not in _CACHE:
        _CACHE['nc'] = _build_nc()
    nc = _CACHE['nc']

    try:
        res = run_bass_kernel_spmd(nc, in_maps, list(range(B)))
    except Exception:
        # transient NRT_EXEC_UNIT_UNRECOVERABLE observed on back-to-back
        # runs; a single retry is free and often clears it
        res = run_bass_kernel_spmd(nc, in_maps, list(range(B)))
    out = np.stack([np.asarray(res.results[i]['y'], np.float32).reshape(C, H, W)
                    for i in range(B)])
    return out


# revision 24
# speedup vs baseline: 1.6847x; 1.1132x over previous
"""Trainium2 Bass kernel for the dynamic-attention-block CNN (nn_DAB).

Data-parallel over batch: 8 samples -> 8 NeuronCores. Each core runs the
full per-sample network with activations resident in SBUF as padded
"frames": 128 partitions = 64 channels x 2 image halves, each half a
98x196 zero-padded row-major frame (rows -1..96 / 95..192 of the 192x192
image, cols -2..193).

Conv structure (all single 128-partition matmuls; the two image halves
ride in one instruction via block-diagonal weights):
  - 3x3 convs and dynamic depthwise convs run in fp8e4m3 DoubleRow mode:
    taps are processed in pairs (lhsT [128,2,128], rhs [128,2,N] with the
    pair dim striding between the two tap offsets), 5 passes per conv.
    Weights are pre-scaled by 16 (convs) / 64 (dw) to dodge fp8
    subnormals; the inverse scale is folded into the engine op that
    drains PSUM.
  - 1x1 convs + channel-attention gates run in bf16: the x*att residual
    is an extra diagonal-matrix matmul accumulating into the same PSUM
    group, so no vector-engine gating pass exists at all.
  - The additive 32x32-upsampled map is folded into conv2 as one fp8
    DoubleRow matmul pass (stride-0 pair, second slot zero): 18
    partitions hold the 9 tap-shifted copies of the upsampled map for
    each half, weights are the channel-summed conv2 taps.

The five stages are software-pipelined ACROSS stage boundaries: stage k
visits tiles in an order rotated by 2(k-1), and the halo rows (the only
cross-half data) are DMA'd as soon as their source tiles (47 and 0)
drain, so the next stage's convs never wait on the previous stage's
tail. Engine balance per tile: PE does all matmuls; DVE drains the two
dw-conv PSUMs (prelu, scale folded into the following 1x1 weights) and
conv2's PSUM; Act drains the two da outputs, conv1 and conv3; GPSIMD
makes conv2's fp8 frame copy.
"""

import sys

for _p in ("/opt/trn_rl_repo", "/root/.axon_site/_ro/pypackages"):
    if _p not in sys.path:
        sys.path.insert(0, _p)

import numpy as np
import ml_dtypes

BF16 = ml_dtypes.bfloat16
F8 = ml_dtypes.float8_e4m3

B, C, H, W = 8, 64, 192, 192
HW = H * W
FR, FC = 98, 196          # frame rows / cols per half
FF = FR * FC              # frame elems per partition
ALPHA = 0.1               # leaky slope
WS = 16.0                 # fp8 conv weight pre-scale
KS = 16.0                 # fp8 dw kernel pre-scale (also bounds fp8 t1)
SC = 256.0                # da-stage PSUM scale: psB = SC*(1x1 out + att*x)

# bias pack columns (the *W columns are pre-scaled by WS / SC for DVE
# drains, which add the bias before the descale multiply; Act drains
# scale first)
(BI_B1, BI_B1W, BI_B2, BI_B2W, BI_B3, BI_B3W,
 BI_CB1, BI_CB1S, BI_CB2, BI_CB2S) = range(10)

# DoubleRow tap pairing: (tap_a, tap_b) with taps t = 3*dy + dx,
# delta(t) = (dy-1)*FC + (dx-1).  5 passes cover all 9 taps; the last
# pass's second slot has zero weight (stride 0 keeps the read in-bounds).
PAIRS = [(0, 1), (3, 4), (6, 7), (2, 5), (8, None)]

# packed fp8 weight slots in wpack8 (kd first: needed at da1 tile 0;
# cw slots are [*, 2, 128] stride-0 DoubleRow pairs appended at the end)
WP_KD1, WP_W1, WP_W2, WP_KD2, WP_W3 = range(5)
WP_CW1, WP_CW2 = 25, 26   # row offsets of the two 1x1 DR pairs
# packed bf16 weight slots in wpackb
WB_G1, WB_G2, WB_RID = range(3)

NTILE = 48
ROT = 2                   # per-stage tile-order rotation

_CACHE = {}


def _delta(t):
    return (t // 3 - 1) * FC + (t % 3 - 1)


def _build_nc():
    import concourse.bacc as bacc
    import concourse.mybir as mybir
    from concourse import tile

    f32 = mybir.dt.float32
    bf16 = mybir.dt.bfloat16
    f8 = mybir.dt.float8e4
    AF = mybir.ActivationFunctionType
    ALU = mybir.AluOpType
    DR = mybir.MatmulPerfMode.DoubleRow

    nc = bacc.Bacc("TRN2", target_bir_lowering=False, debug=False, num_devices=8)

    xb_d = nc.dram_tensor("xb", [128, FF], bf16, kind="ExternalInput").ap()
    x8_d = nc.dram_tensor("x8", [128, FF], f8, kind="ExternalInput").ap()
    wp8_d = nc.dram_tensor("wp8", [128, 27, 2, 128], f8, kind="ExternalInput").ap()
    wpb_d = nc.dram_tensor("wpb", [128, 3, 128], bf16, kind="ExternalInput").ap()
    wa_d = nc.dram_tensor("wa", [18, 2, 128], f8, kind="ExternalInput").ap()
    af_d = nc.dram_tensor("af", [18, FF], f8, kind="ExternalInput").ap()
    bias_d = nc.dram_tensor("bias", [128, 10], f32, kind="ExternalInput").ap()
    y_d = nc.dram_tensor("y", [C, HW], f32, kind="ExternalOutput").ap()

    # row-pair sweep tiles: 48 tiles of 2 image rows (392 frame cols)
    qt = [((2 * i + 1) * FC + 2, 2 * FC) for i in range(NTILE)]

    from contextlib import ExitStack
    with tile.TileContext(nc) as tc, ExitStack() as ctx:
        wpool = ctx.enter_context(tc.tile_pool(name="w", bufs=1))
        fbpool = ctx.enter_context(tc.tile_pool(name="fb", bufs=2))
        f8pool = ctx.enter_context(tc.tile_pool(name="f8", bufs=3))
        t1p = ctx.enter_context(tc.tile_pool(name="t1", bufs=4))
        tmpp = ctx.enter_context(tc.tile_pool(name="tmp", bufs=4))
        outp = ctx.enter_context(tc.tile_pool(name="outp", bufs=4))
        psA = ctx.enter_context(tc.tile_pool(name="psA", bufs=4, space="PSUM"))
        psB = ctx.enter_context(tc.tile_pool(name="psB", bufs=4, space="PSUM"))

        # ---- weights / constants to SBUF (batched DMAs) ----
        wp8 = wpool.tile([128, 27, 2, 128], f8, tag="wp8")
        wpb = wpool.tile([128, 3, 128], bf16, tag="wpb")
        wa = wpool.tile([18, 2, 128], f8, tag="wa")
        af = wpool.tile([18, FF], f8, tag="af")
        bias = wpool.tile([128, 10], f32, tag="bias")

        def w8(slot):
            return wp8[:, 5 * slot:5 * slot + 5, :, :]

        def wcw(row):
            return wp8[:, row, :, :]

        def wb(slot):
            return wpb[:, slot, :]

        def cold(col):
            return bias[:, col:col + 1]

        # ---- input frames (host pre-padded); all bulk input DMAs ride
        # one queue in consumption order: weights first, then x8/xb
        # chunks interleaved by row coverage so da1 never outruns the
        # stream (transfers serialize on the DMA engines, so order is
        # what matters; the af map is only needed from conv2 onward) ----
        Xb = fbpool.tile([128, FF], bf16, tag="fb")
        X8 = f8pool.tile([128, FF], f8, tag="f8")
        O1 = f8pool.tile([128, FF], f8, tag="f8")
        O2 = f8pool.tile([128, FF], f8, tag="f8")
        O3b = fbpool.tile([128, FF], bf16, tag="fb")
        nc.sync.dma_start(out=wp8[...], in_=wp8_d)
        nc.gpsimd.dma_start(out=wpb[...], in_=wpb_d)
        nc.gpsimd.dma_start(out=wa[...], in_=wa_d)
        nc.gpsimd.dma_start(out=bias[...], in_=bias_d)
        nchunk = 8
        step = (FF + nchunk - 1) // nchunk
        for k in range(nchunk):
            c0, c1 = k * step, min((k + 1) * step, FF)
            nc.sync.dma_start(out=X8[:, c0:c1], in_=x8_d[:, c0:c1])
            nc.sync.dma_start(out=Xb[:, c0:c1], in_=xb_d[:, c0:c1])
        nc.scalar.dma_start(out=af[...], in_=af_d)

        # ---- PE warmup: throwaway matmuls while the input DMAs stream
        # in; keeps the p-state ramp finished before real work ----
        wrm = wpool.tile([128, 512], bf16, tag="wrm")
        nc.vector.memset(wrm[:, :], 0.0)
        pw = psA.tile([128, 512], f32, tag="psA")
        for _ in range(14):
            nc.tensor.matmul(pw[:, :], wrm[:, 0:128], wrm[:, :],
                             start=True, stop=True, skip_group_check=True)

        def v3(m):
            return m[:, :].rearrange("p (a b) -> p a b", b=FC)

        # one-time pad zeroing for frame buffers not filled by host DMA.
        # Interior writes never touch pads again, so pads stay zero across
        # all later reuses of these pool buffers.
        for m in (O1, O2, O3b):
            mv = v3(m)
            nc.gpsimd.memset(mv[0:64, 0, :], 0.0)
            nc.gpsimd.memset(mv[64:128, FR - 1, :], 0.0)
            nc.gpsimd.memset(mv[:, :, 0:2], 0.0)
            nc.gpsimd.memset(mv[:, :, FC - 2:FC], 0.0)

        def halo_a(m):
            # half1 top halo row (img 95) <- half0 frame row 96, src tile 47
            mv = v3(m)
            nc.gpsimd.dma_start(out=mv[64:128, 0, :], in_=mv[0:64, 96, :])

        def halo_b(m):
            # half0 bottom halo row (img 96) <- half1 frame row 1, src tile 0
            mv = v3(m)
            nc.gpsimd.dma_start(out=mv[0:64, FR - 1, :], in_=mv[64:128, 1, :])

        def maybe_halo(t, frames):
            if t == 47:
                for m in frames:
                    halo_a(m)
            elif t == 0:
                for m in frames:
                    halo_b(m)

        def order(stage):
            s = (ROT * stage) % NTILE
            return [(s + i) % NTILE for i in range(NTILE)]

        def dr_rhs(m8, q, n, pair):
            ta, tb = pair
            base = q + _delta(ta)
            stride = 0 if tb is None else _delta(tb) - _delta(ta)
            n = min(n, FF - base - max(stride, 0))
            r = m8[:, base:base + 1].copy()
            r.ap[1] = [stride, 2]
            r.ap.append([1, n])
            return r, n

        def conv_dr(ps, wsb, m8, q, n, stop=True):
            # P0 (top-left taps) never clamps, so it is the start pass and
            # always covers the full tile; clamped later passes only lose
            # tail columns that are pad positions, never emitted.
            for p in range(5):
                rhs, np_ = dr_rhs(m8, q, n, PAIRS[p])
                nc.tensor.matmul(ps[:, :np_], wsb[:, p, :, :], rhs,
                                 start=(p == 0), stop=(stop and p == 4),
                                 perf_mode=DR, skip_group_check=True)

        def iview(dst, q):
            # interior-only view: rows of the pair, cols 2:194
            r = q // FC
            return v3(dst)[:, r:r + 2, 2:194]

        def pview(src_ps, n):
            return src_ps[:, :n].rearrange("p (a b) -> p a b", b=FC)[:, :, 0:192]

        # Per-stage drain-engine alternation: each stage's per-engine drain
        # rate must stay below the PE rate or the drain queue backlog
        # throttles PSUM-buffer reuse (and the next stage's drains behind
        # it in the same queue).  Act drains are 1 op (~511ns); DVE prelu
        # drains are 2 ops (~800ns) since scalar_tensor_tensor can't read
        # PSUM, but plain identity+bias drains are 1 DVE op (~533ns).

        def prelu_drain_dve(dst, ps, q, n, bw_col, scale):
            # ((psum + S*b) * 1/S) on DVE (bf16 staging), then an
            # SBUF-only prelu into the frame (scalar_tensor_tensor
            # cannot read PSUM)
            tm = tmpp.tile([128, 2 * FC], bf16, tag="tm")
            tv = tm[:, :n].rearrange("p (a b) -> p a b", b=FC)[:, :, 0:192]
            nc.vector.tensor_scalar(tv, pview(ps, n), cold(bw_col),
                                    1.0 / scale, op0=ALU.add, op1=ALU.mult)
            nc.vector.scalar_tensor_tensor(iview(dst, q), tv, ALPHA, tv,
                                           op0=ALU.mult, op1=ALU.max)

        def da_stage(stage, inb, in8, kd_slot, cw_row, g_slot, cb_col,
                     cbs_col, out8):
            # software-pipelined by two tiles: PE issues dw(j), dw(j+1)
            # before the 1x1+gate of tile j-1 so the in-order PE queue
            # rides out the psA->Act t1->cw dependency chain (~1us).
            kd, cw, g = w8(kd_slot), wcw(cw_row), wb(g_slot)

            def tail(prev, i):
                t1, q, n, t = prev
                pb = psB.tile([128, 2 * FC], f32, tag="psB")
                r = t1[:, 0:1].copy()
                r.ap[1] = [0, 2]
                r.ap.append([1, n])
                nc.tensor.matmul(pb[:, :n], cw, r, start=True, stop=False,
                                 perf_mode=DR, skip_group_check=True)
                nc.tensor.matmul(pb[:, :n], g, inb[:, q:q + n],
                                 start=False, stop=True, skip_group_check=True)
                if i % 4 == 0:
                    nc.scalar.activation(iview(out8, q), pview(pb, n),
                                         AF.Prelu, scale=1.0 / SC,
                                         bias=cold(cb_col), alpha=ALPHA)
                else:
                    prelu_drain_dve(out8, pb, q, n, cbs_col, SC)
                maybe_halo(t, (out8,))

            pipe = []
            for i, t in enumerate(order(stage)):
                q, n = qt[t]
                pa = psA.tile([128, 2 * FC], f32, tag="psA")
                conv_dr(pa, kd, in8, q, n)
                if len(pipe) == 2:
                    tail(pipe.pop(0), i)
                # t1 = prelu(psA) in fp8; the KS dw-weight scale rides
                # along (prelu is positively homogeneous, KS=16 keeps the
                # scaled values inside fp8 range) and is divided out of
                # the fp8 1x1 weights on the host.
                t1 = t1p.tile([128, 2 * FC], f8, tag="t1")
                nc.scalar.activation(t1[:, :n], pa[:, :n], AF.Prelu,
                                     alpha=ALPHA)
                pipe.append((t1, q, n, t))
            for i, prev in enumerate(pipe):
                tail(prev, NTILE + i)

        # ---- network ----
        da_stage(0, Xb, X8, WP_KD1, WP_CW1, WB_G1, BI_CB1, BI_CB1S, O1)

        # conv1 -> prelu -> fp8 frame
        for i, t in enumerate(order(1)):
            q, n = qt[t]
            pa = psA.tile([128, 2 * FC], f32, tag="psA")
            conv_dr(pa, w8(WP_W1), O1, q, n)
            if i % 5 < 3:
                nc.scalar.activation(iview(O2, q), pview(pa, n), AF.Prelu,
                                     scale=1.0 / WS, bias=cold(BI_B1),
                                     alpha=ALPHA)
            else:
                prelu_drain_dve(O2, pa, q, n, BI_B1W, WS)
            maybe_halo(t, (O2,))

        # conv2 (+ additive map as a stride-0 fp8 DR pass) -> identity+bias
        # -> bf16 frame + fp8 copy
        O38 = f8pool.tile([128, FF], f8, tag="f8")
        for i, t in enumerate(order(2)):
            q, n = qt[t]
            pa = psA.tile([128, 2 * FC], f32, tag="psA")
            conv_dr(pa, w8(WP_W2), O2, q, n, stop=False)
            r = af[:, q:q + 1].copy()
            r.ap[1] = [0, 2]
            r.ap.append([1, n])
            nc.tensor.matmul(pa[:, :n], wa[:, :, :], r,
                             start=False, stop=True, perf_mode=DR,
                             skip_group_check=True)
            if i % 2 == 0:
                nc.scalar.activation(iview(O3b, q), pview(pa, n), AF.Identity,
                                     scale=1.0 / WS, bias=cold(BI_B2))
                nc.gpsimd.tensor_copy(O38[:, q:q + n], O3b[:, q:q + n])
            else:
                nc.vector.tensor_scalar(iview(O3b, q), pview(pa, n),
                                        cold(BI_B2W), 1.0 / WS,
                                        op0=ALU.add, op1=ALU.mult)
                nc.vector.tensor_copy(O38[:, q:q + n], O3b[:, q:q + n])
            maybe_halo(t, (O38,))

        O4 = f8pool.tile([128, FF], f8, tag="f8")
        da_stage(3, O3b, O38, WP_KD2, WP_CW2, WB_G2, BI_CB2, BI_CB2S, O4)

        # ---- conv3 + residual: x (bf16, scaled by WS via diag weights)
        # and WS*b3 accumulate straight into PSUM; drain with 1/WS.
        # Output rides in 4-tile (8-row) groups so each group is just two
        # DMAs, both on the otherwise-idle sync queue (HWDGE): per-DMA
        # issue costs would otherwise throttle the tail of the pipeline.
        # order(4) starts at tile 8 (4-aligned), so groups of 4 successive
        # positions cover 4 consecutive tiles even across the wrap. ----
        ot = None
        for j, t in enumerate(order(4)):
            q, n = qt[t]
            pa = psA.tile([128, 2 * FC], f32, tag="psA")
            conv_dr(pa, w8(WP_W3), O4, q, n, stop=False)
            nc.tensor.matmul(pa[:, :n], wb(WB_RID), Xb[:, q:q + n],
                             start=False, stop=True, skip_group_check=True)
            k = j % 4
            if k == 0:
                ot = outp.tile([128, 8, 192], f32, tag="ot")
                g0 = q // FC - 1  # image row of the group's first pair
            otv = ot[:, 2 * k:2 * k + 2, :]
            if j % 2 == 0:
                nc.scalar.activation(otv, pview(pa, n), AF.Identity,
                                     scale=1.0 / WS, bias=cold(BI_B3))
            else:
                nc.vector.tensor_scalar(otv, pview(pa, n),
                                        cold(BI_B3W), 1.0 / WS,
                                        op0=ALU.add, op1=ALU.mult)
            if k == 3:
                nc.sync.dma_start(
                    out=y_d[:, g0 * 192:(g0 + 8) * 192]
                    .rearrange("p (r c) -> p r c", c=192),
                    in_=ot[0:64, :, :])
                nc.sync.dma_start(
                    out=y_d[:, (96 + g0) * 192:(96 + g0 + 8) * 192]
                    .rearrange("p (r c) -> p r c", c=192),
                    in_=ot[64:128, :, :])

    nc.compile()
    return nc


def _pad_frame(xb, dtype):
    """(64,192,192) fp32 -> (128, FR*FC) dual-half padded frame."""
    fr = np.zeros((128, FR, FC), np.float32)
    fr[0:64, 1:97, 2:194] = xb[:, 0:96, :]
    fr[0:64, 97, 2:194] = xb[:, 96, :]
    fr[64:128, 1:97, 2:194] = xb[:, 96:192, :]
    fr[64:128, 0, 2:194] = xb[:, 95, :]
    return np.ascontiguousarray(fr.reshape(128, FF)).astype(dtype)


def _leaky_np(v):
    return np.where(v >= 0, v, ALPHA * v)


def _host_precompute(x, d, p):
    """Build per-core input maps. p: dict of raw weight arrays."""
    d = d.astype(np.float64)
    kern = {}
    att = {}
    for i in (1, 2):
        kw1, kw2 = p[f'da{i}_kw1'].astype(np.float64), p[f'da{i}_kw2'].astype(np.float64)
        ca1, ca2 = p[f'da{i}_ca1'].astype(np.float64), p[f'da{i}_ca2'].astype(np.float64)
        kern[i] = _leaky_np(d @ kw1.T) @ kw2.T          # (B, 576) [c*9+t]
        z = _leaky_np(d @ ca1.T) @ ca2.T
        att[i] = 1.0 / (1.0 + np.exp(-z))               # (B, 64)
    a32 = _leaky_np(d @ p['add_w1'].astype(np.float64).T) @ \
        p['add_w2'].astype(np.float64).T                # (B, 1024)

    cidx = np.arange(128) % 64

    def convw_dr(w):
        # (O, C, 3, 3) fp32 -> [128, 5, 2, 128] f8 block-diag DoubleRow taps
        wq = (w.astype(np.float32) * WS).astype(F8).astype(np.float32)
        wt = wq.transpose(1, 2, 3, 0).reshape(64, 9, 64)  # [c, t, o]
        out = np.zeros((128, 5, 2, 128), np.float32)
        for pi, (ta, tb) in enumerate(PAIRS):
            blk = np.zeros((64, 2, 64), np.float32)
            blk[:, 0, :] = wt[:, ta, :]
            if tb is not None:
                blk[:, 1, :] = wt[:, tb, :]
            out[0:64, pi, :, 0:64] = blk
            out[64:128, pi, :, 64:128] = blk
        return out.astype(F8)

    def cw_dr(w):
        # (O, C) -> [128, 2, 128] fp8 stride-0 DoubleRow pair: slot 0 is
        # the block-diag 1x1 weight scaled by SC/KS (t1 carries KS, the
        # drain divides SC back out), slot 1 is zero.
        out = np.zeros((128, 2, 128), np.float32)
        out[0:64, 0, 0:64] = w.T * (SC / KS)
        out[64:128, 0, 64:128] = w.T * (SC / KS)
        return out.astype(F8)

    # fp8 packed conv/dw weights (per-sample kd slots filled below)
    w1 = convw_dr(p['conv1_w'])
    w2 = convw_dr(p['conv2_w'])
    w3 = convw_dr(p['conv3_w'])
    cw1 = cw_dr(p['da1_cw'])
    cw2 = cw_dr(p['da2_cw'])

    # additive-map conv weights: wa[(h,t), 0, o_col] = WS * sum_c w2[o,c,t]
    w2sum = p['conv2_w'].astype(np.float64).sum(axis=1).reshape(64, 9)  # [o, t]
    wa = np.zeros((18, 2, 128), np.float32)
    for h in range(2):
        for t in range(9):
            wa[h * 9 + t, 0, h * 64:(h + 1) * 64] = WS * w2sum[:, t]
    wa = wa.astype(F8)

    rid = _diag128(np.full(128, WS, np.float32))

    maps = []
    for b in range(B):
        kd = {}
        for i in (1, 2):
            kc = (kern[i][b].reshape(64, 9).astype(np.float32) * KS) \
                .astype(F8).astype(np.float32)           # [c, t]
            kdl = np.zeros((128, 5, 2, 128), np.float32)
            for pi, (ta, tb) in enumerate(PAIRS):
                kdl[np.arange(128), pi, 0, np.arange(128)] = kc[cidx, ta]
                if tb is not None:
                    kdl[np.arange(128), pi, 1, np.arange(128)] = kc[cidx, tb]
            kd[i] = kdl.astype(F8)
        g = {i: _diag128(SC * att[i][b][cidx]) for i in (1, 2)}
        wp8 = np.concatenate(
            [kd[1], w1, w2, kd[2], w3,
             cw1.reshape(128, 1, 2, 128), cw2.reshape(128, 1, 2, 128)],
            axis=1)
        wpb = np.stack([g[1], g[2], rid], axis=1).astype(BF16)
        bias = np.zeros((128, 10), np.float32)
        bias[:, BI_B1] = p['conv1_b'][cidx]
        bias[:, BI_B1W] = WS * p['conv1_b'][cidx]
        bias[:, BI_B2] = p['conv2_b'][cidx]
        bias[:, BI_B2W] = WS * p['conv2_b'][cidx]
        bias[:, BI_B3] = p['conv3_b'][cidx]
        bias[:, BI_B3W] = WS * p['conv3_b'][cidx]
        bias[:, BI_CB1] = p['da1_cb'][cidx]
        bias[:, BI_CB1S] = SC * p['da1_cb'][cidx]
        bias[:, BI_CB2] = p['da2_cb'][cidx]
        bias[:, BI_CB2S] = SC * p['da2_cb'][cidx]

        # additive map frames: 18 partitions = 2 halves x 9 tap shifts
        a = a32[b].astype(np.float32).reshape(32, 32)
        aup = a[np.arange(192) // 6][:, np.arange(192) // 6]  # (192,192)
        afr = np.zeros((2, FF), np.float32)
        fr0 = np.zeros((FR, FC), np.float32)
        fr0[1:97, 2:194] = aup[0:96]
        fr0[97, 2:194] = aup[96]
        afr[0] = fr0.reshape(FF)
        fr1 = np.zeros((FR, FC), np.float32)
        fr1[1:97, 2:194] = aup[96:192]
        fr1[0, 2:194] = aup[95]
        afr[1] = fr1.reshape(FF)
        af = np.zeros((18, FF), np.float32)
        for h in range(2):
            for t in range(9):
                dlt = _delta(t)
                src = afr[h]
                dst = np.zeros(FF, np.float32)
                if dlt >= 0:
                    dst[:FF - dlt] = src[dlt:]
                else:
                    dst[-dlt:] = src[:FF + dlt]
                af[h * 9 + t] = dst
        maps.append(dict(
            xb=_pad_frame(x[b], BF16),
            x8=_pad_frame(x[b], F8),
            wp8=np.ascontiguousarray(wp8).astype(F8),
            wpb=np.ascontiguousarray(wpb),
            wa=np.ascontiguousarray(wa),
            af=np.ascontiguousarray(af).astype(F8),
            bias=bias))
    return maps


def _diag128(v):
    out = np.zeros((128, 128), np.float32)
    out[np.arange(128), np.arange(128)] = v
    return out


def kernel(**inputs):
    from concourse.bass_utils import run_bass_kernel_spmd

    x = np.asarray(inputs['x'], np.float32)
    d = np.asarray(inputs['d'], np.float32)
    in_maps = _host_precompute(x, d, inputs)

    if 'nc' not in _CACHE:
        _CACHE['nc'] = _build_nc()
    nc = _CACHE['nc']

    try:
        res = run_bass_kernel_spmd(nc, in_maps, list(range(B)))
    except Exception:
        # transient NRT_EXEC_UNIT_UNRECOVERABLE observed on back-to-back
        # runs; a single retry is free and often clears it
        res = run_bass_kernel_spmd(nc, in_maps, list(range(B)))
    out = np.stack([np.asarray(res.results[i]['y'], np.float32).reshape(C, H, W)
                    for i in range(B)])
    return out
